# revision 13
# baseline (speedup 1.0000x reference)
"""Trainium2 Bass kernel for a BERT encoder block (single-head attention + FFN).

Sharding: data-parallel over batch — B=8 batches across 8 NeuronCores, one
batch element per core. No collectives.

v2: fp8 (e4m3) DoubleRow attention. q/k/v are projected in bf16 (their fp32
results are kernel outputs), then requantized to scaled fp8. Scores and
attn*v contract two 128-k-tiles per PE instruction (DoubleRow) at 2x bf16
throughput. The softmax denominator is reduced on the PE with an fp8 ones
vector and never divides anything: LayerNorm is invariant to per-column
scaling, so phase B forms h~ = (S_V*den)*x + attn_psum and LN1 absorbs den.
FFN stays bf16 (fp8 would breach the error budget). All matmuls stream
N=512 moving columns (the v projection streamed N=128 in v1).
"""

import sys

if "/opt/trn_rl_repo" not in sys.path:
    sys.path.insert(0, "/opt/trn_rl_repo")

from contextlib import ExitStack

import ml_dtypes
import numpy as np

import concourse.bass as bass
import concourse.tile as tile
from concourse import bacc, bass_utils, mybir
from concourse.bass_isa import ReduceOp

F32 = mybir.dt.float32
BF16 = mybir.dt.bfloat16
F8 = mybir.dt.float8e4
AF = mybir.ActivationFunctionType
OP = mybir.AluOpType
DR = mybir.MatmulPerfMode.DoubleRow

B = 8
S = 2048
E = 1024
F = 4096
P = 128
SBA = 512  # seq block width, qkv projection phase
SBB = 512  # seq block width, attention + LN1 phase
SBC = 512  # seq block width, FFN + LN2 phase
EC = E // P  # 8
FC = F // P  # 32
EPS = 1e-5
SCALE = 1.0 / float(np.sqrt(E))
S_QK = 16.0  # fp8 scale for q/k tiles
S_E = 8.0    # fp8 scale for exp tiles
S_V = 16.0   # fp8 scale for v tiles

_BUILD_CACHE = {}


def _emit(tc, aps, s_total, trivial):
    nc = tc.nc
    NT = s_total // P

    with ExitStack() as outer:
        # ---- constants & small shared pools -------------------------------
        consts = outer.enter_context(tc.tile_pool(name="consts", bufs=1))
        eps1 = consts.tile([P, 1], F32, tag="eps1")
        nc.vector.memset(eps1[:], EPS)
        lnse = consts.tile([P, 1], F32, tag="lnse")
        nc.vector.memset(lnse[:], float(np.log(S_E)))
        ones8 = consts.tile([P, 1], F8, tag="ones8")
        nc.vector.memset(ones8[:], 1.0)
        onesv = consts.tile([1, P], F32, tag="onesv")
        nc.vector.memset(onesv[:], S_V)
        ones1 = consts.tile([1, P], F32, tag="ones1")
        nc.vector.memset(ones1[:], 1.0)
        onesr = consts.tile([P, 1], F32, tag="onesr")
        nc.vector.memset(onesr[:], 1.0)

        def chunked_bias(name, src_ap, nchunk):
            t = consts.tile([P, nchunk], F32, tag=name, name=name)
            nc.sync.dma_start(t[:], src_ap.rearrange("(c p) -> p c", p=P))
            return t

        bq_sb = chunked_bias("bq", aps["bq"], EC)
        bk_sb = chunked_bias("bk", aps["bk"], EC)
        b1_sb = chunked_bias("b1", aps["b1"], FC)
        b2_sb = chunked_bias("b2", aps["b2"], EC)
        g1_sb = chunked_bias("g1", aps["gamma1"], EC)
        g2_sb = chunked_bias("g2", aps["gamma2"], EC)
        # scaled copies of bq/bk for the fp8 requantized q/k tiles
        bq16 = consts.tile([P, EC], F32, tag="bq16")
        nc.vector.tensor_scalar_mul(bq16[:], bq_sb[:], S_QK)
        bk16 = consts.tile([P, EC], F32, tag="bk16")
        nc.vector.tensor_scalar_mul(bk16[:], bk_sb[:], S_QK)
        be1_sb = be2_sb = None
        bv_bc = consts.tile([P, E], F32, tag="bv_bc")
        if not trivial:
            be1_sb = chunked_bias("be1", aps["beta1"], EC)
            be2_sb = chunked_bias("be2", aps["beta2"], EC)
            bv_ap = aps["bv"]
            nc.sync.dma_start(
                bv_bc[:],
                bass.AP(tensor=bv_ap.tensor, offset=bv_ap.offset, ap=[[0, P]] + list(bv_ap.ap)),
            )
        else:
            nc.vector.memset(bv_bc[:], 0.0)

        # `late` pools are opened at phase-B start (so phase A has the SBUF)
        # but released only at the very end (they serve phases B and C).
        late = outer.enter_context(ExitStack())

        stack_ab = outer.enter_context(ExitStack())
        ab = stack_ab.enter_context(tc.tile_pool(name="ab", bufs=1))
        # fp8 operand layouts keep each DoubleRow k-tile pair contiguous
        # (s3_lw_dual_fp8 ISA restriction): [.., pair-idx, 2, tail].
        NBB = s_total // SBB
        qtb = ab.tile([P, NBB, EC // 2, 2, SBB], F8, tag="qtb")
        ktb = ab.tile([P, NT, EC // 2, 2, P], F8, tag="ktb")
        vb = ab.tile([P, NT // 2, EC, 2, P], F8, tag="vb")
        xtb_sb = ab.tile([P, EC, s_total], BF16, tag="xtb")

        # ================= Phase A: q/k/v projections ======================
        with ExitStack() as pha:
            evf = pha.enter_context(tc.tile_pool(name="evf", bufs=6))

            NBA = s_total // SBA
            # blocked host layout: one 1MB DMA per seq block with 8KB
            # contiguous lines (strided 1KB lines gated kernel start in v2)
            for pz in range(NBA):
                zsl = slice(pz * SBA, (pz + 1) * SBA)
                nc.sync.dma_start(
                    xtb_sb[:, :, zsl],
                    aps["xtb"][pz].rearrange("p (c s) -> p c s", c=EC),
                )

            # --- qT and kT: packed weights [m][p][c][j] streamed per m ---
            with tc.tile_pool(name="wqk", bufs=3) as wqk_pool, \
                 tc.tile_pool(name="ps_qk", bufs=8, space="PSUM") as ps_qk:
                for w_ap, b_sb, bsc, o_ap, tb in (
                    (aps["wq"], bq_sb, bq16, aps["qt_o"], qtb),
                    (aps["wk"], bk_sb, bk16, aps["kt_o"], ktb),
                ):
                    for m in range(EC):
                        wt = wqk_pool.tile([P, EC, P], BF16, tag="wqk", name="wqk")
                        nc.sync.dma_start(
                            wt[:], w_ap[m].rearrange("p (c j) -> p c j", j=P)
                        )
                        pts = []
                        for blk in range(NBA):
                            pts.append(ps_qk.tile([P, SBA], F32, tag="projps", name="projps"))
                        for c in range(EC):
                            for blk in range(NBA):
                                nc.tensor.matmul(
                                    pts[blk][:],
                                    wt[:, c, :],
                                    xtb_sb[:, c, blk * SBA : (blk + 1) * SBA],
                                    start=(c == 0),
                                    stop=(c == EC - 1),
                                )
                        for blk in range(NBA):
                            f32t = evf.tile([P, SBA], F32, tag="evf", name="evf")
                            nc.vector.tensor_scalar_add(f32t[:], pts[blk][:], b_sb[:, m : m + 1])
                            if tb is qtb:
                                f8dst = qtb[:, blk, m // 2, m % 2, :]
                            else:
                                f8dst = ktb[:, 4 * blk : 4 * (blk + 1), m // 2, m % 2, :]
                            nc.scalar.activation(
                                f8dst,
                                pts[blk][:],
                                AF.Identity,
                                bias=bsc[:, m : m + 1],
                                scale=S_QK,
                            )
                            nc.sync.dma_start(
                                o_ap[m * P : (m + 1) * P, blk * SBA : (blk + 1) * SBA], f32t[:]
                            )

            # wv packed [p][(c, m, j)]: moving slices of 512 e-columns
            wv_pool = pha.enter_context(tc.tile_pool(name="wvp", bufs=1))
            wv_sb = wv_pool.tile([P, EC, E], BF16, tag="wv")
            nc.sync.dma_start(
                wv_sb[:], aps["wv"].rearrange("p (c n) -> p c n", c=EC)
            )

            # --- v natural: xT stationary, wv moving with N=512 ---
            with tc.tile_pool(name="ps_v", bufs=3, space="PSUM", side="right") as ps_v:
                for st in range(NT):
                    for eb in range(E // SBA):
                        esl = slice(eb * SBA, (eb + 1) * SBA)
                        vp = ps_v.tile([P, SBA], F32, tag="vps", name="vps")
                        for c in range(EC):
                            nc.tensor.matmul(
                                vp[:],
                                xtb_sb[:, c, st * P : (st + 1) * P],
                                wv_sb[:, c, esl],
                                start=(c == 0),
                                stop=(c == EC - 1),
                            )
                        f32t = evf.tile([P, SBA], F32, tag="evf", name="evf")
                        nc.vector.tensor_add(f32t[:], vp[:], bv_bc[:, esl])
                        nc.scalar.activation(
                            vb[:, st // 2, 4 * eb : 4 * (eb + 1), st % 2, :],
                            f32t[:], AF.Identity, scale=S_V,
                        )
                        nc.sync.dma_start(aps["v_o"][st * P : (st + 1) * P, esl], f32t[:])

        # ================= Phase B: attention + LN1 ========================
        hln = late.enter_context(tc.tile_pool(name="hln", bufs=1, side="right"))
        hlnt = hln.tile([P, EC, s_total], BF16, tag="hlnt")
        bc_pool = late.enter_context(tc.tile_pool(name="bc", bufs=2, side="right"))
        tmp_pool = late.enter_context(tc.tile_pool(name="tmp", bufs=3, side="right"))
        sq_pool = late.enter_context(tc.tile_pool(name="sq", bufs=2, side="right"))

        def ln_finish(acc_h, acc_sq, n, w):
            """Partition-allreduce raw sums (gpsimd), then mean/rstd [P, w]."""
            nc.gpsimd.partition_all_reduce(acc_h[:], acc_h[:], P, ReduceOp.add)
            nc.gpsimd.partition_all_reduce(acc_sq[:], acc_sq[:], P, ReduceOp.add)
            mean_bc = bc_pool.tile([P, w], F32, tag="mean_bc", name="mean_bc", bufs=1)
            nc.vector.tensor_scalar_mul(mean_bc[:], acc_h[:], 1.0 / n)
            var_bc = bc_pool.tile([P, w], F32, tag="var_bc", name="var_bc", bufs=1)
            nc.vector.tensor_scalar_mul(var_bc[:], acc_sq[:], 1.0 / n)
            m2 = tmp_pool.tile([P, w], F32, tag="lnsub", name="m2")
            nc.vector.tensor_mul(m2[:], mean_bc[:], mean_bc[:])
            nc.vector.tensor_sub(var_bc[:], var_bc[:], m2[:])
            nc.scalar.activation(var_bc[:], var_bc[:], AF.Sqrt, bias=eps1[:])
            rstd_bc = bc_pool.tile([P, w], F32, tag="rstd_bc", name="rstd_bc", bufs=1)
            nc.vector.reciprocal(rstd_bc[:], var_bc[:])
            return mean_bc, rstd_bc

        def ln_apply_one(eng, src, mean_bc, rstd_bc, g_sb, be_sb2, c, dst, w):
            t1 = tmp_pool.tile([P, w], F32, tag="lnsub", name="lnsub")
            eng.tensor_sub(t1[:], src[:], mean_bc[:])
            if trivial:
                eng.tensor_mul(dst, t1[:], rstd_bc[:])
            else:
                eng.scalar_tensor_tensor(
                    dst, t1[:], g_sb[:, c : c + 1], rstd_bc[:], op0=OP.mult, op1=OP.mult
                )
                eng.tensor_scalar_add(dst, dst, be_sb2[:, c : c + 1])

        with ExitStack() as phb:
            pb = phb.enter_context(tc.tile_pool(name="pb", bufs=1))
            hpre_pool = phb.enter_context(tc.tile_pool(name="hpre", bufs=16))
            den_pool = phb.enter_context(tc.tile_pool(name="den", bufs=2))
            ps_sc = phb.enter_context(tc.tile_pool(name="ps_sc", bufs=2, space="PSUM"))
            ps_av = phb.enter_context(tc.tile_pool(name="ps_av", bufs=3, space="PSUM"))
            ps_dn = phb.enter_context(tc.tile_pool(name="ps_dn", bufs=1, space="PSUM"))
            ps_db = phb.enter_context(tc.tile_pool(name="ps_db", bufs=2, space="PSUM"))

            def ln1_finish_apply(pend):
                p_acc_h, p_acc_sq, p_hp, p_ssl = pend
                mean_bc, rstd_bc = ln_finish(p_acc_h, p_acc_sq, float(E), SBB)
                for c in range(EC):
                    ln_apply_one(
                        nc.vector, p_hp[c], mean_bc, rstd_bc, g1_sb, be1_sb, c,
                        hlnt[:, c, p_ssl], SBB,
                    )

            pend = None
            for blk in range(s_total // SBB):
                ssl = slice(blk * SBB, (blk + 1) * SBB)
                exp_sb = pb.tile([P, NT, SBB], F8, tag="exp", name="exp", bufs=2)
                # --- scoresT -> exp (fp8, *S_E via ln-bias) ---
                for t in range(NT):
                    sp = ps_sc.tile([P, SBB], F32, tag="scps", name="scps")
                    for cp in range(EC // 2):
                        nc.tensor.matmul(
                            sp[:],
                            ktb[:, t, cp, :, :],
                            qtb[:, blk, cp, :, :],
                            start=(cp == 0),
                            stop=(cp == EC // 2 - 1),
                            perf_mode=DR,
                        )
                    nc.scalar.activation(
                        exp_sb[:, t, :], sp[:], AF.Exp,
                        scale=SCALE / (S_QK * S_QK), bias=lnse[:],
                    )
                # --- den = sum_t sum_p exp8 on the PE (ones vector) ---
                denp = ps_dn.tile([1, SBB], F32, tag="denp", name="denp")
                for t in range(NT):
                    nc.tensor.matmul(
                        denp[:],
                        ones8[:],
                        exp_sb[:, t, :],
                        start=(t == 0),
                        stop=(t == NT - 1),
                    )
                den_sb = den_pool.tile([1, SBB], F32, tag="den_sb", name="den_sb")
                nc.scalar.copy(den_sb[:], denp[:])
                # broadcast S_V*den to all partitions (fp32 matmul)
                denbc = ps_db.tile([P, SBB], F32, tag="denbc", name="denbc")
                nc.tensor.matmul(denbc[:], onesv[:], den_sb[:], start=True, stop=True)

                # previous block's LN1 tail: emitted here so its gpsimd/DVE work
                # runs under this block's scores, freeing hpre tiles before the
                # next block's evictions need them.
                if pend is not None:
                    ln1_finish_apply(pend)
                    pend = None

                # --- attn_outT; h~ = S_V*den*x + av (no division: LN absorbs) ---
                acc_h = bc_pool.tile([P, SBB], F32, tag="acc_h", name="acc_h")
                acc_sq = bc_pool.tile([P, SBB], F32, tag="acc_sq", name="acc_sq")
                hp_tiles = []
                for et in range(EC):
                    avp = ps_av.tile([P, SBB], F32, tag="avps", name="avps")
                    for tp in range(NT // 2):
                        nc.tensor.matmul(
                            avp[:],
                            vb[:, tp, et, :, :],
                            exp_sb[:, 2 * tp : 2 * tp + 2, :],
                            start=(tp == 0),
                            stop=(tp == NT // 2 - 1),
                            perf_mode=DR,
                        )
                    t1 = tmp_pool.tile([P, SBB], F32, tag="resid", name="resid")
                    nc.vector.tensor_mul(t1[:], denbc[:], xtb_sb[:, et, ssl])
                    hp = hpre_pool.tile([P, SBB], F32, tag="hpre", name="hpre")
                    nc.vector.tensor_add(hp[:], avp[:], t1[:])
                    hp_tiles.append(hp)
                    sq = sq_pool.tile([P, SBB], F32, tag="sq", name="sq")
                    nc.scalar.activation(sq[:], hp[:], AF.Square)
                    if et == 0:
                        nc.vector.tensor_copy(acc_h[:], hp[:])
                        nc.gpsimd.tensor_copy(acc_sq[:], sq[:])
                    else:
                        nc.vector.tensor_add(acc_h[:], acc_h[:], hp[:])
                        nc.gpsimd.tensor_add(acc_sq[:], acc_sq[:], sq[:])
                pend = (acc_h, acc_sq, hp_tiles, ssl)
            ln1_finish_apply(pend)

        # q/k/v fp8 tiles and xtb are dead after phase B — release their SBUF.
        stack_ab.close()

        # ================= Phase C: FFN + LN2 ==============================
        with ExitStack() as phc:
            pc = phc.enter_context(tc.tile_pool(name="pc", bufs=1))
            w1_pool = phc.enter_context(tc.tile_pool(name="w1p", bufs=3))
            w2_pool = phc.enter_context(tc.tile_pool(name="w2p", bufs=2))
            oev_pool = phc.enter_context(tc.tile_pool(name="oev", bufs=3))
            opre_pool = phc.enter_context(tc.tile_pool(name="opre", bufs=11))
            mm_ps = phc.enter_context(ExitStack())
            ps_f1 = mm_ps.enter_context(tc.tile_pool(name="ps_f1", bufs=4, space="PSUM"))
            ps_f2 = mm_ps.enter_context(tc.tile_pool(name="ps_f2", bufs=3, space="PSUM"))

            relu_sb = pc.tile([P, 2 * FC, SBC], BF16, tag="relu")
            NBC = s_total // SBC

            def ln2_finish_apply(pend):
                p_acc_h, p_acc_sq, p_op, p_ssl = pend
                mean2_bc, rstd2_bc = ln_finish(p_acc_h, p_acc_sq, float(E), SBC)
                for c in range(EC):
                    ot = oev_pool.tile([P, SBC], F32, tag="oev", name="oev")
                    ln_apply_one(nc.vector, p_op[c], mean2_bc, rstd2_bc, g2_sb, be2_sb, c, ot[:], SBC)
                    nc.sync.dma_start(aps["outt_o"][c * P : (c + 1) * P, p_ssl], ot[:])

            def ln2_finish_apply_pe(pend, ps_tail):
                """Last-block tail: stats reduce/broadcast on the PE, apply
                split across DVE and GpSimd — shortens the end-of-kernel
                serial chain (no gpsimd allreduce latency)."""
                p_acc_h, p_acc_sq, p_op, p_ssl = pend
                stp_h = ps_tail.tile([1, SBC], F32, tag="stph", name="stp_h", bufs=1)
                stp_q = ps_tail.tile([1, SBC], F32, tag="stpq", name="stp_q", bufs=1)
                nc.tensor.matmul(stp_h[:], onesr[:], p_acc_h[:], start=True, stop=True)
                nc.tensor.matmul(stp_q[:], onesr[:], p_acc_sq[:], start=True, stop=True)
                pk = den_pool2.tile([1, 2 * SBC], F32, tag="pk", name="pk")
                mean1 = pk[:, 0:SBC]
                rstd1 = pk[:, SBC : 2 * SBC]
                nc.vector.tensor_scalar_mul(mean1, stp_h[:], 1.0 / float(E))
                nc.vector.tensor_scalar_mul(rstd1, stp_q[:], 1.0 / float(E))
                m2 = den_pool2.tile([1, SBC], F32, tag="m2", name="m2")
                nc.vector.tensor_mul(m2[:], mean1, mean1)
                nc.vector.tensor_sub(rstd1, rstd1, m2[:])
                nc.scalar.activation(rstd1, rstd1, AF.Sqrt, bias=eps1[0:1, :])
                nc.vector.reciprocal(rstd1, rstd1)
                mean_bc = ps_tail.tile([P, SBC], F32, tag="mbc", name="mbc", bufs=1)
                rstd_bc = ps_tail.tile([P, SBC], F32, tag="rbc", name="rbc", bufs=1)
                nc.tensor.matmul(mean_bc[:], ones1[:], mean1, start=True, stop=True)
                nc.tensor.matmul(rstd_bc[:], ones1[:], rstd1, start=True, stop=True)
                mean_sb = bc_pool.tile([P, SBC], F32, tag="mean_bc", name="mean_sb", bufs=1)
                rstd_sb = bc_pool.tile([P, SBC], F32, tag="rstd_bc", name="rstd_sb", bufs=1)
                nc.scalar.copy(mean_sb[:], mean_bc[:])
                nc.scalar.copy(rstd_sb[:], rstd_bc[:])
                for c in range(EC):
                    eng = nc.vector if c < 6 else nc.gpsimd
                    ot = oev_pool.tile([P, SBC], F32, tag="oev", name="oev")
                    ln_apply_one(eng, p_op[c], mean_sb, rstd_sb, g2_sb, be2_sb, c, ot[:], SBC)
                    nc.sync.dma_start(aps["outt_o"][c * P : (c + 1) * P, p_ssl], ot[:])

            den_pool2 = phc.enter_context(tc.tile_pool(name="den2", bufs=1))

            pend2 = None
            for blk in range(NBC):
                ssl = slice(blk * SBC, (blk + 1) * SBC)
                # --- ffn1 + relu (W1 streamed in 512-col groups) ---
                for fg in range(F // SBC):
                    wt = w1_pool.tile([P, EC, SBC], BF16, tag="w1", name="w1t")
                    nc.sync.dma_start(
                        wt[:], aps["w1"][fg].rearrange("p (c j) -> p c j", c=EC)
                    )
                    for fi in range(SBC // P):
                        ft = fg * (SBC // P) + fi
                        fp = ps_f1.tile([P, SBC], F32, tag="f1ps", name="f1ps")
                        for c in range(EC):
                            nc.tensor.matmul(
                                fp[:],
                                wt[:, c, fi * P : (fi + 1) * P],
                                hlnt[:, c, ssl],
                                start=(c == 0),
                                stop=(c == EC - 1),
                            )
                        nc.scalar.activation(
                            relu_sb[:, (blk % 2) * FC + ft, :],
                            fp[:], AF.Relu, bias=b1_sb[:, ft : ft + 1]
                        )

                if pend2 is not None:
                    ln2_finish_apply(pend2)
                    pend2 = None

                # --- ffn2 + b2 + residual; stats on gpsimd ---
                acc2_h = bc_pool.tile([P, SBC], F32, tag="acc_h", name="acc2_h")
                acc2_sq = bc_pool.tile([P, SBC], F32, tag="acc_sq", name="acc2_sq")
                op_tiles = []
                for et in range(EC):
                    w2t = w2_pool.tile([P, FC, P], BF16, tag="w2t", name="w2t")
                    nc.sync.dma_start(
                        w2t[:], aps["w2"][et].rearrange("p (f j) -> p f j", j=P)
                    )
                    op_ps = ps_f2.tile([P, SBC], F32, tag="f2ps", name="f2ps")
                    for f in range(FC):
                        nc.tensor.matmul(
                            op_ps[:],
                            w2t[:, f, :],
                            relu_sb[:, (blk % 2) * FC + f, :],
                            start=(f == 0),
                            stop=(f == FC - 1),
                        )
                    opc = opre_pool.tile([P, SBC], F32, tag="opre", name="opre")
                    nc.vector.scalar_tensor_tensor(
                        opc[:],
                        op_ps[:],
                        b2_sb[:, et : et + 1],
                        hlnt[:, et, ssl],
                        op0=OP.add,
                        op1=OP.add,
                    )
                    op_tiles.append(opc)
                    sq = sq_pool.tile([P, SBC], F32, tag="sq", name="sq")
                    nc.scalar.activation(sq[:], opc[:], AF.Square)
                    if et == 0:
                        nc.vector.tensor_copy(acc2_h[:], opc[:])
                        nc.gpsimd.tensor_copy(acc2_sq[:], sq[:])
                    else:
                        nc.vector.tensor_add(acc2_h[:], acc2_h[:], opc[:])
                        nc.gpsimd.tensor_add(acc2_sq[:], acc2_sq[:], sq[:])
                pend2 = (acc2_h, acc2_sq, op_tiles, ssl)
            mm_ps.close()
            with tc.tile_pool(name="ps_tail", bufs=1, space="PSUM") as ps_tail:
                ln2_finish_apply_pe(pend2, ps_tail)


def _build(s_total, trivial):
    key = (s_total, trivial)
    if key in _BUILD_CACHE:
        return _BUILD_CACHE[key]
    nc = bacc.Bacc("TRN2", target_bir_lowering=False, debug=False, num_devices=B)
    aps = {}

    def din(name, shape, dt):
        aps[name] = nc.dram_tensor(name, shape, dt, kind="ExternalInput").ap()

    def dout(name, shape, dt):
        aps[name] = nc.dram_tensor(name, shape, dt, kind="ExternalOutput").ap()

    din("xtb", [s_total // SBA, P, EC * SBA], BF16)
    din("wq", [EC, P, E], BF16)        # [m][p][(c, j)] packed
    din("wk", [EC, P, E], BF16)
    din("wv", [P, EC * E], BF16)       # [p][(c, m, j)] packed
    din("w1", [F // SBC, P, EC * SBC], BF16)  # [fg][p][(c, j)] packed
    din("w2", [EC, P, F], BF16)        # [et][p][(f, j)] packed
    for nm, n in (("bq", E), ("bk", E), ("bv", E), ("b1", F), ("b2", E),
                  ("gamma1", E), ("beta1", E), ("gamma2", E), ("beta2", E)):
        din(nm, [n], F32)
    dout("qt_o", [E, s_total], F32)
    dout("kt_o", [E, s_total], F32)
    dout("v_o", [s_total, E], F32)
    dout("outt_o", [E, s_total], F32)

    with tile.TileContext(nc) as tc:
        _emit(tc, aps, s_total, trivial)
    nc.compile()
    _BUILD_CACHE[key] = (nc, aps)
    return nc, aps


def _pack_qk(w):
    # out[m, p, c*128+j] = w[c*128+p, m*128+j]
    return np.ascontiguousarray(
        w.reshape(EC, P, EC, P).transpose(2, 1, 0, 3).reshape(EC, P, E)
    )


def _pack_wv(w):
    # out[p, (c, m, j)] = w[c*128+p, m*128+j]
    return np.ascontiguousarray(
        w.reshape(EC, P, EC, P).transpose(1, 0, 2, 3).reshape(P, EC * E)
    )


def _pack_w1(w):
    # out[fg, p, c*512+j] = w1[c*128+p, fg*512+j]
    return np.ascontiguousarray(
        w.reshape(EC, P, F // SBC, SBC).transpose(2, 1, 0, 3).reshape(F // SBC, P, EC * SBC)
    )


def _pack_w2(w):
    # out[et, p, (f, j)] = w[f*128+p, et*128+j]
    return np.ascontiguousarray(
        w.reshape(FC, P, EC, P).transpose(2, 1, 0, 3).reshape(EC, P, F)
    )


def _prep_in_maps(inputs):
    bf = ml_dtypes.bfloat16
    x = np.ascontiguousarray(np.asarray(inputs["x"], dtype=np.float32))
    shared = {
        "wq": _pack_qk(np.asarray(inputs["Wq"], np.float32).astype(bf)),
        "wk": _pack_qk(np.asarray(inputs["Wk"], np.float32).astype(bf)),
        "wv": _pack_wv(np.asarray(inputs["Wv"], np.float32).astype(bf)),
        "w1": _pack_w1(np.asarray(inputs["W1"], np.float32).astype(bf)),
        "w2": _pack_w2(np.asarray(inputs["W2"], np.float32).astype(bf)),
    }
    for nm in ("bq", "bk", "bv", "b1", "b2", "gamma1", "beta1", "gamma2", "beta2"):
        shared[nm] = np.asarray(inputs[nm], np.float32)
    in_maps = []
    for b in range(x.shape[0]):
        m = dict(shared)
        xt = x[b].T.astype(bf)  # [E, S]
        m["xtb"] = np.ascontiguousarray(
            xt.reshape(EC, P, S // SBA, SBA).transpose(2, 1, 0, 3).reshape(S // SBA, P, EC * SBA)
        )
        in_maps.append(m)
    return in_maps


def kernel(**inputs):
    trivial = all(
        not np.any(np.asarray(inputs[k])) for k in ("bv", "beta1", "beta2")
    ) and all(np.all(np.asarray(inputs[k]) == 1.0) for k in ("gamma1", "gamma2"))
    nc, _ = _build(S, trivial)
    in_maps = _prep_in_maps(inputs)
    res = bass_utils.run_bass_kernel_spmd(nc, in_maps, core_ids=list(range(B)))
    out = np.empty((B, S, E), np.float32)
    q = np.empty((B, S, E), np.float32)
    k = np.empty((B, S, E), np.float32)
    v = np.empty((B, S, E), np.float32)
    for b in range(B):
        r = res.results[b]
        out[b] = r["outt_o"].T
        q[b] = r["qt_o"].T
        k[b] = r["kt_o"].T
        v[b] = r["v_o"]
    return (out, k, q, v)


# revision 16
# speedup vs baseline: 1.0315x; 1.0315x over previous
"""Trainium2 Bass kernel for a BERT encoder block (single-head attention + FFN).

Sharding: data-parallel over batch — B=8 batches across 8 NeuronCores, one
batch element per core. No collectives.

v2: fp8 (e4m3) DoubleRow attention. q/k/v are projected in bf16 (their fp32
results are kernel outputs), then requantized to scaled fp8. Scores and
attn*v contract two 128-k-tiles per PE instruction (DoubleRow) at 2x bf16
throughput. The softmax denominator is reduced on the PE with an fp8 ones
vector and never divides anything: LayerNorm is invariant to per-column
scaling, so phase B forms h~ = (S_V*den)*x + attn_psum and LN1 absorbs den.
FFN stays bf16 (fp8 would breach the error budget). All matmuls stream
N=512 moving columns (the v projection streamed N=128 in v1).
"""

import sys

if "/opt/trn_rl_repo" not in sys.path:
    sys.path.insert(0, "/opt/trn_rl_repo")

from contextlib import ExitStack

import ml_dtypes
import numpy as np

import concourse.bass as bass
import concourse.tile as tile
from concourse import bacc, bass_utils, mybir
from concourse.bass_isa import ReduceOp

F32 = mybir.dt.float32
BF16 = mybir.dt.bfloat16
F8 = mybir.dt.float8e4
AF = mybir.ActivationFunctionType
OP = mybir.AluOpType
DR = mybir.MatmulPerfMode.DoubleRow

B = 8
S = 2048
E = 1024
F = 4096
P = 128
SBA = 512  # seq block width, qkv projection phase
SBB = 512  # seq block width, attention + LN1 phase
SBC = 512  # seq block width, FFN + LN2 phase
EC = E // P  # 8
FC = F // P  # 32
EPS = 1e-5
SCALE = 1.0 / float(np.sqrt(E))
S_QK = 16.0  # fp8 scale for q/k tiles
S_E = 8.0    # fp8 scale for exp tiles
S_V = 16.0   # fp8 scale for v tiles

_BUILD_CACHE = {}


def _emit(tc, aps, s_total, trivial):
    nc = tc.nc
    NT = s_total // P

    with ExitStack() as outer:
        # ---- constants & small shared pools -------------------------------
        consts = outer.enter_context(tc.tile_pool(name="consts", bufs=1))
        eps1 = consts.tile([P, 1], F32, tag="eps1")
        nc.vector.memset(eps1[:], EPS)
        lnse = consts.tile([P, 1], F32, tag="lnse")
        nc.vector.memset(lnse[:], float(np.log(S_E)))
        ones8 = consts.tile([P, 1], F8, tag="ones8")
        nc.vector.memset(ones8[:], 1.0)
        onesv = consts.tile([1, P], F32, tag="onesv")
        nc.vector.memset(onesv[:], S_V)
        ones1 = consts.tile([1, P], F32, tag="ones1")
        nc.vector.memset(ones1[:], 1.0)
        onesr = consts.tile([P, 1], F32, tag="onesr")
        nc.vector.memset(onesr[:], 1.0)

        def chunked_bias(name, src_ap, nchunk):
            t = consts.tile([P, nchunk], F32, tag=name, name=name)
            nc.sync.dma_start(t[:], src_ap.rearrange("(c p) -> p c", p=P))
            return t

        bq_sb = chunked_bias("bq", aps["bq"], EC)
        bk_sb = chunked_bias("bk", aps["bk"], EC)
        b1_sb = chunked_bias("b1", aps["b1"], FC)
        b2_sb = chunked_bias("b2", aps["b2"], EC)
        g1_sb = chunked_bias("g1", aps["gamma1"], EC)
        g2_sb = chunked_bias("g2", aps["gamma2"], EC)
        # scaled copies of bq/bk for the fp8 requantized q/k tiles
        bq16 = consts.tile([P, EC], F32, tag="bq16")
        nc.vector.tensor_scalar_mul(bq16[:], bq_sb[:], S_QK)
        bk16 = consts.tile([P, EC], F32, tag="bk16")
        nc.vector.tensor_scalar_mul(bk16[:], bk_sb[:], S_QK)
        be1_sb = be2_sb = None
        bv_bc = consts.tile([P, E], F32, tag="bv_bc")
        if not trivial:
            be1_sb = chunked_bias("be1", aps["beta1"], EC)
            be2_sb = chunked_bias("be2", aps["beta2"], EC)
            bv_ap = aps["bv"]
            nc.sync.dma_start(
                bv_bc[:],
                bass.AP(tensor=bv_ap.tensor, offset=bv_ap.offset, ap=[[0, P]] + list(bv_ap.ap)),
            )
        else:
            nc.vector.memset(bv_bc[:], 0.0)

        # `late` pools are opened at phase-B start (so phase A has the SBUF)
        # but released only at the very end (they serve phases B and C).
        late = outer.enter_context(ExitStack())

        stack_ab = outer.enter_context(ExitStack())
        ab = stack_ab.enter_context(tc.tile_pool(name="ab", bufs=1))
        # fp8 operand layouts keep each DoubleRow k-tile pair contiguous
        # (s3_lw_dual_fp8 ISA restriction): [.., pair-idx, 2, tail].
        NBB = s_total // SBB
        qtb = ab.tile([P, NBB, EC // 2, 2, SBB], F8, tag="qtb")
        ktb = ab.tile([P, NT, EC // 2, 2, P], F8, tag="ktb")
        vb = ab.tile([P, NT // 2, EC, 2, P], F8, tag="vb")
        xtb_sb = ab.tile([P, EC, s_total], BF16, tag="xtb")

        # ================= Phase A: q/k/v projections ======================
        with ExitStack() as pha:
            evf = pha.enter_context(tc.tile_pool(name="evf", bufs=6))

            NBA = s_total // SBA
            # blocked host layout: one 1MB DMA per seq block with 8KB
            # contiguous lines (strided 1KB lines gated kernel start in v2)
            for pz in range(NBA):
                zsl = slice(pz * SBA, (pz + 1) * SBA)
                nc.sync.dma_start(
                    xtb_sb[:, :, zsl],
                    aps["xtb"][pz].rearrange("p (c s) -> p c s", c=EC),
                )

            # --- qT and kT: packed weights [m][p][c][j] streamed per m ---
            with tc.tile_pool(name="wqk", bufs=3) as wqk_pool, \
                 tc.tile_pool(name="ps_qk", bufs=8, space="PSUM") as ps_qk:
                for w_ap, b_sb, bsc, o_ap, tb in (
                    (aps["wq"], bq_sb, bq16, aps["qt_o"], qtb),
                    (aps["wk"], bk_sb, bk16, aps["kt_o"], ktb),
                ):
                    for m in range(EC):
                        wt = wqk_pool.tile([P, EC, P], BF16, tag="wqk", name="wqk")
                        nc.sync.dma_start(
                            wt[:], w_ap[m].rearrange("p (c j) -> p c j", j=P)
                        )
                        pts = []
                        for blk in range(NBA):
                            pts.append(ps_qk.tile([P, SBA], F32, tag="projps", name="projps"))
                        for c in range(EC):
                            for blk in range(NBA):
                                nc.tensor.matmul(
                                    pts[blk][:],
                                    wt[:, c, :],
                                    xtb_sb[:, c, blk * SBA : (blk + 1) * SBA],
                                    start=(c == 0),
                                    stop=(c == EC - 1),
                                )
                        for blk in range(NBA):
                            f32t = evf.tile([P, SBA], F32, tag="evf", name="evf")
                            nc.vector.tensor_scalar_add(f32t[:], pts[blk][:], b_sb[:, m : m + 1])
                            if tb is qtb:
                                f8dst = qtb[:, blk, m // 2, m % 2, :]
                            else:
                                f8dst = ktb[:, 4 * blk : 4 * (blk + 1), m // 2, m % 2, :]
                            nc.scalar.activation(
                                f8dst,
                                pts[blk][:],
                                AF.Identity,
                                bias=bsc[:, m : m + 1],
                                scale=S_QK,
                            )
                            nc.sync.dma_start(
                                o_ap[m * P : (m + 1) * P, blk * SBA : (blk + 1) * SBA], f32t[:]
                            )

            # wv packed [p][(c, m, j)]: moving slices of 512 e-columns
            wv_pool = pha.enter_context(tc.tile_pool(name="wvp", bufs=1))
            wv_sb = wv_pool.tile([P, EC, E], BF16, tag="wv")
            nc.sync.dma_start(
                wv_sb[:], aps["wv"].rearrange("p (c n) -> p c n", c=EC)
            )

            # --- v natural: xT stationary, wv moving with N=512 ---
            with tc.tile_pool(name="ps_v", bufs=3, space="PSUM", side="right") as ps_v:
                for st in range(NT):
                    for eb in range(E // SBA):
                        esl = slice(eb * SBA, (eb + 1) * SBA)
                        vp = ps_v.tile([P, SBA], F32, tag="vps", name="vps")
                        for c in range(EC):
                            nc.tensor.matmul(
                                vp[:],
                                xtb_sb[:, c, st * P : (st + 1) * P],
                                wv_sb[:, c, esl],
                                start=(c == 0),
                                stop=(c == EC - 1),
                            )
                        f32t = evf.tile([P, SBA], F32, tag="evf", name="evf")
                        nc.vector.tensor_add(f32t[:], vp[:], bv_bc[:, esl])
                        nc.scalar.activation(
                            vb[:, st // 2, 4 * eb : 4 * (eb + 1), st % 2, :],
                            f32t[:], AF.Identity, scale=S_V,
                        )
                        nc.sync.dma_start(aps["v_o"][st * P : (st + 1) * P, esl], f32t[:])

        # ================= Phase B: attention + LN1 ========================
        hln = late.enter_context(tc.tile_pool(name="hln", bufs=1, side="right"))
        hlnt = hln.tile([P, EC, s_total], BF16, tag="hlnt")
        bc_pool = late.enter_context(tc.tile_pool(name="bc", bufs=2, side="right"))
        tmp_pool = late.enter_context(tc.tile_pool(name="tmp", bufs=3, side="right"))
        sq_pool = late.enter_context(tc.tile_pool(name="sq", bufs=2, side="right"))

        def ln_finish(acc_h, acc_sq, n, w):
            """Partition-allreduce raw sums (gpsimd), then mean/rstd [P, w]."""
            nc.gpsimd.partition_all_reduce(acc_h[:], acc_h[:], P, ReduceOp.add)
            nc.gpsimd.partition_all_reduce(acc_sq[:], acc_sq[:], P, ReduceOp.add)
            mean_bc = bc_pool.tile([P, w], F32, tag="mean_bc", name="mean_bc", bufs=1)
            nc.vector.tensor_scalar_mul(mean_bc[:], acc_h[:], 1.0 / n)
            var_bc = bc_pool.tile([P, w], F32, tag="var_bc", name="var_bc", bufs=1)
            nc.vector.tensor_scalar_mul(var_bc[:], acc_sq[:], 1.0 / n)
            m2 = tmp_pool.tile([P, w], F32, tag="lnsub", name="m2")
            nc.vector.tensor_mul(m2[:], mean_bc[:], mean_bc[:])
            nc.vector.tensor_sub(var_bc[:], var_bc[:], m2[:])
            nc.scalar.activation(var_bc[:], var_bc[:], AF.Sqrt, bias=eps1[:])
            rstd_bc = bc_pool.tile([P, w], F32, tag="rstd_bc", name="rstd_bc", bufs=1)
            nc.vector.reciprocal(rstd_bc[:], var_bc[:])
            return mean_bc, rstd_bc

        def ln_apply_one(eng, src, mean_bc, rstd_bc, g_sb, be_sb2, c, dst, w):
            t1 = tmp_pool.tile([P, w], F32, tag="lnsub", name="lnsub")
            eng.tensor_sub(t1[:], src[:], mean_bc[:])
            if trivial:
                eng.tensor_mul(dst, t1[:], rstd_bc[:])
            else:
                eng.scalar_tensor_tensor(
                    dst, t1[:], g_sb[:, c : c + 1], rstd_bc[:], op0=OP.mult, op1=OP.mult
                )
                eng.tensor_scalar_add(dst, dst, be_sb2[:, c : c + 1])

        with ExitStack() as phb:
            pb = phb.enter_context(tc.tile_pool(name="pb", bufs=1))
            hpre_pool = phb.enter_context(tc.tile_pool(name="hpre", bufs=16))
            den_pool = phb.enter_context(tc.tile_pool(name="den", bufs=2))
            ps_sc = phb.enter_context(tc.tile_pool(name="ps_sc", bufs=2, space="PSUM"))
            ps_av = phb.enter_context(tc.tile_pool(name="ps_av", bufs=3, space="PSUM"))
            ps_dn = phb.enter_context(tc.tile_pool(name="ps_dn", bufs=1, space="PSUM"))
            ps_db = phb.enter_context(tc.tile_pool(name="ps_db", bufs=2, space="PSUM"))

            N_GP_APPLY = 2  # LN1 applies on gpsimd (early); rest on DVE (late)

            def ln1_finish_part1(pend):
                p_acc_h, p_acc_sq, p_hp, p_ssl = pend
                mean_bc, rstd_bc = ln_finish(p_acc_h, p_acc_sq, float(E), SBB)
                for c in range(EC - N_GP_APPLY, EC):
                    ln_apply_one(
                        nc.gpsimd, p_hp[c], mean_bc, rstd_bc, g1_sb, be1_sb, c,
                        hlnt[:, c, p_ssl], SBB,
                    )
                return mean_bc, rstd_bc

            def ln1_apply_dve(pend, mean_bc, rstd_bc):
                p_acc_h, p_acc_sq, p_hp, p_ssl = pend
                for c in range(EC - N_GP_APPLY):
                    ln_apply_one(
                        nc.vector, p_hp[c], mean_bc, rstd_bc, g1_sb, be1_sb, c,
                        hlnt[:, c, p_ssl], SBB,
                    )

            pend = None
            for blk in range(s_total // SBB):
                ssl = slice(blk * SBB, (blk + 1) * SBB)
                exp_sb = pb.tile([P, NT, SBB], F8, tag="exp", name="exp", bufs=2)
                # --- scoresT -> exp (fp8, *S_E via ln-bias) ---
                for t in range(NT):
                    sp = ps_sc.tile([P, SBB], F32, tag="scps", name="scps")
                    for cp in range(EC // 2):
                        nc.tensor.matmul(
                            sp[:],
                            ktb[:, t, cp, :, :],
                            qtb[:, blk, cp, :, :],
                            start=(cp == 0),
                            stop=(cp == EC // 2 - 1),
                            perf_mode=DR,
                        )
                    nc.scalar.activation(
                        exp_sb[:, t, :], sp[:], AF.Exp,
                        scale=SCALE / (S_QK * S_QK), bias=lnse[:],
                    )
                # --- den = sum_t sum_p exp8 on the PE (ones vector) ---
                denp = ps_dn.tile([1, SBB], F32, tag="denp", name="denp")
                for t in range(NT):
                    nc.tensor.matmul(
                        denp[:],
                        ones8[:],
                        exp_sb[:, t, :],
                        start=(t == 0),
                        stop=(t == NT - 1),
                    )
                den_sb = den_pool.tile([1, SBB], F32, tag="den_sb", name="den_sb")
                nc.scalar.copy(den_sb[:], denp[:])
                # broadcast S_V*den to all partitions (fp32 matmul)
                denbc = ps_db.tile([P, SBB], F32, tag="denbc", name="denbc")
                nc.tensor.matmul(denbc[:], onesv[:], den_sb[:], start=True, stop=True)

                # previous block's LN1 reduce + gpsimd applies run under this
                # block's scores; its DVE applies are emitted after this block's
                # evictions so they never delay PSUM drain.
                mr = None
                if pend is not None:
                    mr = ln1_finish_part1(pend)

                # --- attn_outT; h~ = S_V*den*x + av (no division: LN absorbs) ---
                acc_h = bc_pool.tile([P, SBB], F32, tag="acc_h", name="acc_h")
                acc_sq = bc_pool.tile([P, SBB], F32, tag="acc_sq", name="acc_sq")
                hp_tiles = []
                for et in range(EC):
                    avp = ps_av.tile([P, SBB], F32, tag="avps", name="avps")
                    for tp in range(NT // 2):
                        nc.tensor.matmul(
                            avp[:],
                            vb[:, tp, et, :, :],
                            exp_sb[:, 2 * tp : 2 * tp + 2, :],
                            start=(tp == 0),
                            stop=(tp == NT // 2 - 1),
                            perf_mode=DR,
                        )
                    t1 = tmp_pool.tile([P, SBB], F32, tag="resid", name="resid")
                    nc.vector.tensor_mul(t1[:], denbc[:], xtb_sb[:, et, ssl])
                    hp = hpre_pool.tile([P, SBB], F32, tag="hpre", name="hpre")
                    nc.vector.tensor_add(hp[:], avp[:], t1[:])
                    hp_tiles.append(hp)
                    sq = sq_pool.tile([P, SBB], F32, tag="sq", name="sq")
                    nc.scalar.activation(sq[:], hp[:], AF.Square)
                    if et == 0:
                        nc.vector.tensor_copy(acc_h[:], hp[:])
                        nc.gpsimd.tensor_copy(acc_sq[:], sq[:])
                    else:
                        nc.vector.tensor_add(acc_h[:], acc_h[:], hp[:])
                        nc.gpsimd.tensor_add(acc_sq[:], acc_sq[:], sq[:])
                if pend is not None:
                    ln1_apply_dve(pend, *mr)
                pend = (acc_h, acc_sq, hp_tiles, ssl)
            mr = ln1_finish_part1(pend)
            ln1_apply_dve(pend, *mr)

        # q/k/v fp8 tiles and xtb are dead after phase B — release their SBUF.
        stack_ab.close()

        # ================= Phase C: FFN + LN2 ==============================
        with ExitStack() as phc:
            pc = phc.enter_context(tc.tile_pool(name="pc", bufs=1))
            w1_pool = phc.enter_context(tc.tile_pool(name="w1p", bufs=3))
            w2_pool = phc.enter_context(tc.tile_pool(name="w2p", bufs=2))
            oev_pool = phc.enter_context(tc.tile_pool(name="oev", bufs=3))
            opre_pool = phc.enter_context(tc.tile_pool(name="opre", bufs=18))
            mm_ps = phc.enter_context(ExitStack())
            ps_f1 = mm_ps.enter_context(tc.tile_pool(name="ps_f1", bufs=4, space="PSUM"))
            ps_f2 = mm_ps.enter_context(tc.tile_pool(name="ps_f2", bufs=3, space="PSUM"))

            relu_sb = pc.tile([P, 2 * FC, SBC], BF16, tag="relu")
            NBC = s_total // SBC

            def ln2_finish_apply(pend):
                p_acc_h, p_acc_sq, p_op, p_ssl = pend
                mean2_bc, rstd2_bc = ln_finish(p_acc_h, p_acc_sq, float(E), SBC)
                for c in range(EC):
                    ot = oev_pool.tile([P, SBC], F32, tag="oev", name="oev")
                    ln_apply_one(nc.vector, p_op[c], mean2_bc, rstd2_bc, g2_sb, be2_sb, c, ot[:], SBC)
                    nc.sync.dma_start(aps["outt_o"][c * P : (c + 1) * P, p_ssl], ot[:])

            def ln2_finish_apply_pe(pend, ps_tail):
                """Last-block tail: stats reduce/broadcast on the PE, apply
                split across DVE and GpSimd — shortens the end-of-kernel
                serial chain (no gpsimd allreduce latency)."""
                p_acc_h, p_acc_sq, p_op, p_ssl = pend
                stp_h = ps_tail.tile([1, SBC], F32, tag="stph", name="stp_h", bufs=1)
                stp_q = ps_tail.tile([1, SBC], F32, tag="stpq", name="stp_q", bufs=1)
                nc.tensor.matmul(stp_h[:], onesr[:], p_acc_h[:], start=True, stop=True)
                nc.tensor.matmul(stp_q[:], onesr[:], p_acc_sq[:], start=True, stop=True)
                pk = den_pool2.tile([1, 2 * SBC], F32, tag="pk", name="pk")
                mean1 = pk[:, 0:SBC]
                rstd1 = pk[:, SBC : 2 * SBC]
                nc.vector.tensor_scalar_mul(mean1, stp_h[:], 1.0 / float(E))
                nc.vector.tensor_scalar_mul(rstd1, stp_q[:], 1.0 / float(E))
                m2 = den_pool2.tile([1, SBC], F32, tag="m2", name="m2")
                nc.vector.tensor_mul(m2[:], mean1, mean1)
                nc.vector.tensor_sub(rstd1, rstd1, m2[:])
                nc.scalar.activation(rstd1, rstd1, AF.Sqrt, bias=eps1[0:1, :])
                nc.vector.reciprocal(rstd1, rstd1)
                mean_bc = ps_tail.tile([P, SBC], F32, tag="mbc", name="mbc", bufs=1)
                rstd_bc = ps_tail.tile([P, SBC], F32, tag="rbc", name="rbc", bufs=1)
                nc.tensor.matmul(mean_bc[:], ones1[:], mean1, start=True, stop=True)
                nc.tensor.matmul(rstd_bc[:], ones1[:], rstd1, start=True, stop=True)
                mean_sb = bc_pool.tile([P, SBC], F32, tag="mean_bc", name="mean_sb", bufs=1)
                rstd_sb = bc_pool.tile([P, SBC], F32, tag="rstd_bc", name="rstd_sb", bufs=1)
                nc.scalar.copy(mean_sb[:], mean_bc[:])
                nc.scalar.copy(rstd_sb[:], rstd_bc[:])
                for c in range(EC):
                    eng = nc.vector if c < 6 else nc.gpsimd
                    ot = oev_pool.tile([P, SBC], F32, tag="oev", name="oev")
                    ln_apply_one(eng, p_op[c], mean_sb, rstd_sb, g2_sb, be2_sb, c, ot[:], SBC)
                    nc.sync.dma_start(aps["outt_o"][c * P : (c + 1) * P, p_ssl], ot[:])

            den_pool2 = phc.enter_context(tc.tile_pool(name="den2", bufs=1))

            def f1_group(blk, ssl, wt, fg):
                for fi in range(SBC // P):
                    ft = fg * (SBC // P) + fi
                    fp = ps_f1.tile([P, SBC], F32, tag="f1ps", name="f1ps")
                    for c in range(EC):
                        nc.tensor.matmul(
                            fp[:],
                            wt[:, c, fi * P : (fi + 1) * P],
                            hlnt[:, c, ssl],
                            start=(c == 0),
                            stop=(c == EC - 1),
                        )
                    nc.scalar.activation(
                        relu_sb[:, (blk % 2) * FC + ft, :],
                        fp[:], AF.Relu, bias=b1_sb[:, ft : ft + 1]
                    )

            def f2_one(blk, ssl, et, w2t, acc2_h, acc2_sq, op_tiles):
                op_ps = ps_f2.tile([P, SBC], F32, tag="f2ps", name="f2ps")
                for f in range(FC):
                    nc.tensor.matmul(
                        op_ps[:],
                        w2t[:, f, :],
                        relu_sb[:, (blk % 2) * FC + f, :],
                        start=(f == 0),
                        stop=(f == FC - 1),
                    )
                opc = opre_pool.tile([P, SBC], BF16, tag="opre", name="opre")
                nc.vector.scalar_tensor_tensor(
                    opc[:],
                    op_ps[:],
                    b2_sb[:, et : et + 1],
                    hlnt[:, et, ssl],
                    op0=OP.add,
                    op1=OP.add,
                )
                op_tiles.append(opc)
                sq = sq_pool.tile([P, SBC], F32, tag="sq", name="sq")
                nc.scalar.activation(sq[:], opc[:], AF.Square)
                if et == 0:
                    nc.vector.tensor_copy(acc2_h[:], opc[:])
                    nc.gpsimd.tensor_copy(acc2_sq[:], sq[:])
                else:
                    nc.vector.tensor_add(acc2_h[:], acc2_h[:], opc[:])
                    nc.gpsimd.tensor_add(acc2_sq[:], acc2_sq[:], sq[:])

            def load_w2(et):
                w2t = w2_pool.tile([P, FC, P], BF16, tag="w2t", name="w2t")
                nc.sync.dma_start(
                    w2t[:], aps["w2"][et].rearrange("p (f j) -> p f j", j=P)
                )
                return w2t

            # Blocks are processed in pairs sharing one W1/W2 streaming pass
            # (v4 re-streamed 16MB per block and saturated the DMA). The last
            # pair runs f2 block-sequential so only one block's LN2 remains
            # for the short PE-stats tail.
            pends = []
            NPAIR = NBC // 2
            pend_last = None
            for bp in range(NPAIR):
                b0, b1 = 2 * bp, 2 * bp + 1
                ssl0 = slice(b0 * SBC, (b0 + 1) * SBC)
                ssl1 = slice(b1 * SBC, (b1 + 1) * SBC)
                for fg in range(F // SBC):
                    wt = w1_pool.tile([P, EC, SBC], BF16, tag="w1", name="w1t")
                    nc.sync.dma_start(
                        wt[:], aps["w1"][fg].rearrange("p (c j) -> p c j", c=EC)
                    )
                    f1_group(b0, ssl0, wt, fg)
                    f1_group(b1, ssl1, wt, fg)
                # pending LN2 tails run under the f1 window (DVE idle there)
                for pd in pends:
                    ln2_finish_apply(pd)
                pends = []
                if bp < NPAIR - 1:
                    acc_a = bc_pool.tile([P, SBC], F32, tag="acc_h", name="acc2_h")
                    acc_asq = bc_pool.tile([P, SBC], F32, tag="acc_sq", name="acc2_sq")
                    acc_b = bc_pool.tile([P, SBC], F32, tag="acc_h", name="acc2_h")
                    acc_bsq = bc_pool.tile([P, SBC], F32, tag="acc_sq", name="acc2_sq")
                    ops_a, ops_b = [], []
                    for et in range(EC):
                        w2t = load_w2(et)
                        f2_one(b0, ssl0, et, w2t, acc_a, acc_asq, ops_a)
                        f2_one(b1, ssl1, et, w2t, acc_b, acc_bsq, ops_b)
                    pends = [(acc_a, acc_asq, ops_a, ssl0), (acc_b, acc_bsq, ops_b, ssl1)]
                else:
                    acc_a = bc_pool.tile([P, SBC], F32, tag="acc_h", name="acc2_h")
                    acc_asq = bc_pool.tile([P, SBC], F32, tag="acc_sq", name="acc2_sq")
                    ops_a = []
                    for et in range(EC):
                        f2_one(b0, ssl0, et, load_w2(et), acc_a, acc_asq, ops_a)
                    acc_b = bc_pool.tile([P, SBC], F32, tag="acc_h", name="acc2_h")
                    acc_bsq = bc_pool.tile([P, SBC], F32, tag="acc_sq", name="acc2_sq")
                    ops_b = []
                    for et in range(EC):
                        if et == 1:
                            # b0's LN2 tail runs under b1's remaining f2 work
                            ln2_finish_apply((acc_a, acc_asq, ops_a, ssl0))
                        f2_one(b1, ssl1, et, load_w2(et), acc_b, acc_bsq, ops_b)
                    pend_last = (acc_b, acc_bsq, ops_b, ssl1)
            mm_ps.close()
            with tc.tile_pool(name="ps_tail", bufs=1, space="PSUM") as ps_tail:
                ln2_finish_apply_pe(pend_last, ps_tail)


def _build(s_total, trivial):
    key = (s_total, trivial)
    if key in _BUILD_CACHE:
        return _BUILD_CACHE[key]
    nc = bacc.Bacc("TRN2", target_bir_lowering=False, debug=False, num_devices=B)
    aps = {}

    def din(name, shape, dt):
        aps[name] = nc.dram_tensor(name, shape, dt, kind="ExternalInput").ap()

    def dout(name, shape, dt):
        aps[name] = nc.dram_tensor(name, shape, dt, kind="ExternalOutput").ap()

    din("xtb", [s_total // SBA, P, EC * SBA], BF16)
    din("wq", [EC, P, E], BF16)        # [m][p][(c, j)] packed
    din("wk", [EC, P, E], BF16)
    din("wv", [P, EC * E], BF16)       # [p][(c, m, j)] packed
    din("w1", [F // SBC, P, EC * SBC], BF16)  # [fg][p][(c, j)] packed
    din("w2", [EC, P, F], BF16)        # [et][p][(f, j)] packed
    for nm, n in (("bq", E), ("bk", E), ("bv", E), ("b1", F), ("b2", E),
                  ("gamma1", E), ("beta1", E), ("gamma2", E), ("beta2", E)):
        din(nm, [n], F32)
    dout("qt_o", [E, s_total], F32)
    dout("kt_o", [E, s_total], F32)
    dout("v_o", [s_total, E], F32)
    dout("outt_o", [E, s_total], F32)

    with tile.TileContext(nc) as tc:
        _emit(tc, aps, s_total, trivial)
    nc.compile()
    _BUILD_CACHE[key] = (nc, aps)
    return nc, aps


def _pack_qk(w):
    # out[m, p, c*128+j] = w[c*128+p, m*128+j]
    return np.ascontiguousarray(
        w.reshape(EC, P, EC, P).transpose(2, 1, 0, 3).reshape(EC, P, E)
    )


def _pack_wv(w):
    # out[p, (c, m, j)] = w[c*128+p, m*128+j]
    return np.ascontiguousarray(
        w.reshape(EC, P, EC, P).transpose(1, 0, 2, 3).reshape(P, EC * E)
    )


def _pack_w1(w):
    # out[fg, p, c*512+j] = w1[c*128+p, fg*512+j]
    return np.ascontiguousarray(
        w.reshape(EC, P, F // SBC, SBC).transpose(2, 1, 0, 3).reshape(F // SBC, P, EC * SBC)
    )


def _pack_w2(w):
    # out[et, p, (f, j)] = w[f*128+p, et*128+j]
    return np.ascontiguousarray(
        w.reshape(FC, P, EC, P).transpose(2, 1, 0, 3).reshape(EC, P, F)
    )


def _prep_in_maps(inputs):
    bf = ml_dtypes.bfloat16
    x = np.ascontiguousarray(np.asarray(inputs["x"], dtype=np.float32))
    shared = {
        "wq": _pack_qk(np.asarray(inputs["Wq"], np.float32).astype(bf)),
        "wk": _pack_qk(np.asarray(inputs["Wk"], np.float32).astype(bf)),
        "wv": _pack_wv(np.asarray(inputs["Wv"], np.float32).astype(bf)),
        "w1": _pack_w1(np.asarray(inputs["W1"], np.float32).astype(bf)),
        "w2": _pack_w2(np.asarray(inputs["W2"], np.float32).astype(bf)),
    }
    for nm in ("bq", "bk", "bv", "b1", "b2", "gamma1", "beta1", "gamma2", "beta2"):
        shared[nm] = np.asarray(inputs[nm], np.float32)
    in_maps = []
    for b in range(x.shape[0]):
        m = dict(shared)
        xt = x[b].T.astype(bf)  # [E, S]
        m["xtb"] = np.ascontiguousarray(
            xt.reshape(EC, P, S // SBA, SBA).transpose(2, 1, 0, 3).reshape(S // SBA, P, EC * SBA)
        )
        in_maps.append(m)
    return in_maps


def kernel(**inputs):
    trivial = all(
        not np.any(np.asarray(inputs[k])) for k in ("bv", "beta1", "beta2")
    ) and all(np.all(np.asarray(inputs[k]) == 1.0) for k in ("gamma1", "gamma2"))
    nc, _ = _build(S, trivial)
    in_maps = _prep_in_maps(inputs)
    res = bass_utils.run_bass_kernel_spmd(nc, in_maps, core_ids=list(range(B)))
    out = np.empty((B, S, E), np.float32)
    q = np.empty((B, S, E), np.float32)
    k = np.empty((B, S, E), np.float32)
    v = np.empty((B, S, E), np.float32)
    for b in range(B):
        r = res.results[b]
        out[b] = r["outt_o"].T
        q[b] = r["qt_o"].T
        k[b] = r["kt_o"].T
        v[b] = r["v_o"]
    return (out, k, q, v)


# revision 19
# speedup vs baseline: 1.0610x; 1.0286x over previous
"""Trainium2 Bass kernel for a BERT encoder block (single-head attention + FFN).

Sharding: data-parallel over batch — B=8 batches across 8 NeuronCores, one
batch element per core. No collectives.

v2: fp8 (e4m3) DoubleRow attention. q/k/v are projected in bf16 (their fp32
results are kernel outputs), then requantized to scaled fp8. Scores and
attn*v contract two 128-k-tiles per PE instruction (DoubleRow) at 2x bf16
throughput. The softmax denominator is reduced on the PE with an fp8 ones
vector and never divides anything: LayerNorm is invariant to per-column
scaling, so phase B forms h~ = (S_V*den)*x + attn_psum and LN1 absorbs den.
FFN stays bf16 (fp8 would breach the error budget). All matmuls stream
N=512 moving columns (the v projection streamed N=128 in v1).
"""

import sys

if "/opt/trn_rl_repo" not in sys.path:
    sys.path.insert(0, "/opt/trn_rl_repo")

from contextlib import ExitStack

import ml_dtypes
import numpy as np

import concourse.bass as bass
import concourse.tile as tile
from concourse import bacc, bass_utils, mybir
from concourse.bass_isa import ReduceOp

F32 = mybir.dt.float32
BF16 = mybir.dt.bfloat16
F8 = mybir.dt.float8e4
AF = mybir.ActivationFunctionType
OP = mybir.AluOpType
DR = mybir.MatmulPerfMode.DoubleRow

B = 8
S = 2048
E = 1024
F = 4096
P = 128
SBA = 512  # seq block width, qkv projection phase
SBB = 512  # seq block width, attention + LN1 phase
SBC = 512  # seq block width, FFN + LN2 phase
EC = E // P  # 8
FC = F // P  # 32
EPS = 1e-5
SCALE = 1.0 / float(np.sqrt(E))
S_QK = 16.0  # fp8 scale for q/k tiles
S_E = 8.0    # fp8 scale for exp tiles
S_V = 16.0   # fp8 scale for v tiles

_BUILD_CACHE = {}


def _emit(tc, aps, s_total, trivial):
    nc = tc.nc
    NT = s_total // P

    with ExitStack() as outer:
        # ---- constants & small shared pools -------------------------------
        consts = outer.enter_context(tc.tile_pool(name="consts", bufs=1))
        eps1 = consts.tile([P, 1], F32, tag="eps1")
        nc.vector.memset(eps1[:], EPS)
        lnse = consts.tile([P, 1], F32, tag="lnse")
        nc.vector.memset(lnse[:], float(np.log(S_E)))
        ones8 = consts.tile([P, 1], F8, tag="ones8")
        nc.vector.memset(ones8[:], 1.0)
        onesv = consts.tile([1, P], F32, tag="onesv")
        nc.vector.memset(onesv[:], S_V)
        ones1 = consts.tile([1, P], F32, tag="ones1")
        nc.vector.memset(ones1[:], 1.0)
        onesr = consts.tile([P, 1], F32, tag="onesr")
        nc.vector.memset(onesr[:], 1.0)

        def chunked_bias(name, src_ap, nchunk):
            t = consts.tile([P, nchunk], F32, tag=name, name=name)
            nc.sync.dma_start(t[:], src_ap.rearrange("(c p) -> p c", p=P))
            return t

        bq_sb = chunked_bias("bq", aps["bq"], EC)
        bk_sb = chunked_bias("bk", aps["bk"], EC)
        b1_sb = chunked_bias("b1", aps["b1"], FC)
        b2_sb = chunked_bias("b2", aps["b2"], EC)
        g1_sb = chunked_bias("g1", aps["gamma1"], EC)
        g2_sb = chunked_bias("g2", aps["gamma2"], EC)
        # scaled copies of bq/bk for the fp8 requantized q/k tiles
        bq16 = consts.tile([P, EC], F32, tag="bq16")
        nc.vector.tensor_scalar_mul(bq16[:], bq_sb[:], S_QK)
        bk16 = consts.tile([P, EC], F32, tag="bk16")
        nc.vector.tensor_scalar_mul(bk16[:], bk_sb[:], S_QK)
        be1_sb = be2_sb = None
        bv_bc = consts.tile([P, E], F32, tag="bv_bc")
        if not trivial:
            be1_sb = chunked_bias("be1", aps["beta1"], EC)
            be2_sb = chunked_bias("be2", aps["beta2"], EC)
            bv_ap = aps["bv"]
            nc.sync.dma_start(
                bv_bc[:],
                bass.AP(tensor=bv_ap.tensor, offset=bv_ap.offset, ap=[[0, P]] + list(bv_ap.ap)),
            )
        else:
            nc.vector.memset(bv_bc[:], 0.0)

        # `late` pools are opened at phase-B start (so phase A has the SBUF)
        # but released only at the very end (they serve phases B and C).
        late = outer.enter_context(ExitStack())

        stack_ab = outer.enter_context(ExitStack())
        ab = stack_ab.enter_context(tc.tile_pool(name="ab", bufs=1))
        # fp8 operand layouts keep each DoubleRow k-tile pair contiguous
        # (s3_lw_dual_fp8 ISA restriction): [.., pair-idx, 2, tail].
        NBB = s_total // SBB
        qtb = ab.tile([P, NBB, EC // 2, 2, SBB], F8, tag="qtb")
        ktb = ab.tile([P, NT, EC // 2, 2, P], F8, tag="ktb")
        vb = ab.tile([P, NT // 2, EC, 2, P], F8, tag="vb")
        xtb_sb = ab.tile([P, EC, s_total], BF16, tag="xtb")

        # ================= Phase A: q/k/v projections ======================
        with ExitStack() as pha:
            evf = pha.enter_context(tc.tile_pool(name="evf", bufs=6))

            NBA = s_total // SBA
            # blocked host layout: one 1MB DMA per seq block with 8KB
            # contiguous lines (strided 1KB lines gated kernel start in v2)
            for pz in range(NBA):
                zsl = slice(pz * SBA, (pz + 1) * SBA)
                nc.sync.dma_start(
                    xtb_sb[:, :, zsl],
                    aps["xtb"][pz].rearrange("p (c s) -> p c s", c=EC),
                )

            # --- qT and kT: packed weights [m][p][c][j] streamed per m ---
            with tc.tile_pool(name="wqk", bufs=3) as wqk_pool, \
                 tc.tile_pool(name="ps_qk", bufs=8, space="PSUM") as ps_qk:
                for w_ap, b_sb, bsc, o_ap, tb in (
                    (aps["wq"], bq_sb, bq16, aps["qt_o"], qtb),
                    (aps["wk"], bk_sb, bk16, aps["kt_o"], ktb),
                ):
                    for m in range(EC):
                        wt = wqk_pool.tile([P, EC, P], BF16, tag="wqk", name="wqk")
                        nc.sync.dma_start(
                            wt[:], w_ap[m].rearrange("p (c j) -> p c j", j=P)
                        )
                        pts = []
                        for blk in range(NBA):
                            pts.append(ps_qk.tile([P, SBA], F32, tag="projps", name="projps"))
                        for c in range(EC):
                            for blk in range(NBA):
                                nc.tensor.matmul(
                                    pts[blk][:],
                                    wt[:, c, :],
                                    xtb_sb[:, c, blk * SBA : (blk + 1) * SBA],
                                    start=(c == 0),
                                    stop=(c == EC - 1),
                                )
                        for blk in range(NBA):
                            f32t = evf.tile([P, SBA], F32, tag="evf", name="evf")
                            nc.vector.tensor_scalar_add(f32t[:], pts[blk][:], b_sb[:, m : m + 1])
                            if tb is qtb:
                                f8dst = qtb[:, blk, m // 2, m % 2, :]
                            else:
                                f8dst = ktb[:, 4 * blk : 4 * (blk + 1), m // 2, m % 2, :]
                            nc.scalar.activation(
                                f8dst,
                                pts[blk][:],
                                AF.Identity,
                                bias=bsc[:, m : m + 1],
                                scale=S_QK,
                            )
                            nc.sync.dma_start(
                                o_ap[m * P : (m + 1) * P, blk * SBA : (blk + 1) * SBA], f32t[:]
                            )

            # wv packed [p][(c, m, j)]: moving slices of 512 e-columns
            wv_pool = pha.enter_context(tc.tile_pool(name="wvp", bufs=1))
            wv_sb = wv_pool.tile([P, EC, E], BF16, tag="wv")
            nc.sync.dma_start(
                wv_sb[:], aps["wv"].rearrange("p (c n) -> p c n", c=EC)
            )

            # --- v natural: xT stationary, wv moving with N=512 ---
            with tc.tile_pool(name="ps_v", bufs=3, space="PSUM", side="right") as ps_v:
                for st in range(NT):
                    for eb in range(E // SBA):
                        esl = slice(eb * SBA, (eb + 1) * SBA)
                        vp = ps_v.tile([P, SBA], F32, tag="vps", name="vps")
                        for c in range(EC):
                            nc.tensor.matmul(
                                vp[:],
                                xtb_sb[:, c, st * P : (st + 1) * P],
                                wv_sb[:, c, esl],
                                start=(c == 0),
                                stop=(c == EC - 1),
                            )
                        f32t = evf.tile([P, SBA], F32, tag="evf", name="evf")
                        nc.vector.tensor_add(f32t[:], vp[:], bv_bc[:, esl])
                        nc.scalar.activation(
                            vb[:, st // 2, 4 * eb : 4 * (eb + 1), st % 2, :],
                            f32t[:], AF.Identity, scale=S_V,
                        )
                        nc.sync.dma_start(aps["v_o"][st * P : (st + 1) * P, esl], f32t[:])

        # ================= Phase B: attention + LN1 ========================
        hln = late.enter_context(tc.tile_pool(name="hln", bufs=1, side="right"))
        hlnt = hln.tile([P, EC, s_total], BF16, tag="hlnt")
        bc_pool = late.enter_context(tc.tile_pool(name="bc", bufs=2, side="right"))
        tmp_pool = late.enter_context(tc.tile_pool(name="tmp", bufs=2, side="right"))
        sq_pool = late.enter_context(tc.tile_pool(name="sq", bufs=2, side="right"))

        def ln_finish(acc_h, acc_sq, n, w):
            """Partition-allreduce raw sums (gpsimd), then mean/rstd [P, w]."""
            nc.gpsimd.partition_all_reduce(acc_h[:], acc_h[:], P, ReduceOp.add)
            nc.gpsimd.partition_all_reduce(acc_sq[:], acc_sq[:], P, ReduceOp.add)
            mean_bc = bc_pool.tile([P, w], F32, tag="mean_bc", name="mean_bc", bufs=1)
            nc.vector.tensor_scalar_mul(mean_bc[:], acc_h[:], 1.0 / n)
            var_bc = bc_pool.tile([P, w], F32, tag="var_bc", name="var_bc", bufs=1)
            nc.vector.tensor_scalar_mul(var_bc[:], acc_sq[:], 1.0 / n)
            m2 = tmp_pool.tile([P, w], F32, tag="lnsub", name="m2")
            nc.vector.tensor_mul(m2[:], mean_bc[:], mean_bc[:])
            nc.vector.tensor_sub(var_bc[:], var_bc[:], m2[:])
            nc.scalar.activation(var_bc[:], var_bc[:], AF.Sqrt, bias=eps1[:])
            rstd_bc = bc_pool.tile([P, w], F32, tag="rstd_bc", name="rstd_bc", bufs=1)
            nc.vector.reciprocal(rstd_bc[:], var_bc[:])
            return mean_bc, rstd_bc

        def ln_apply_one(eng, src, mean_bc, rstd_bc, g_sb, be_sb2, c, dst, w):
            t1 = tmp_pool.tile([P, w], F32, tag="lnsub", name="lnsub")
            eng.tensor_sub(t1[:], src[:], mean_bc[:])
            if trivial:
                eng.tensor_mul(dst, t1[:], rstd_bc[:])
            else:
                eng.scalar_tensor_tensor(
                    dst, t1[:], g_sb[:, c : c + 1], rstd_bc[:], op0=OP.mult, op1=OP.mult
                )
                eng.tensor_scalar_add(dst, dst, be_sb2[:, c : c + 1])

        with ExitStack() as phb:
            pb = phb.enter_context(tc.tile_pool(name="pb", bufs=1))
            hpre_pool = phb.enter_context(tc.tile_pool(name="hpre", bufs=24))
            den_pool = phb.enter_context(tc.tile_pool(name="den", bufs=2))
            ps_sc = phb.enter_context(tc.tile_pool(name="ps_sc", bufs=2, space="PSUM"))
            ps_av = phb.enter_context(tc.tile_pool(name="ps_av", bufs=3, space="PSUM"))
            ps_dn = phb.enter_context(tc.tile_pool(name="ps_dn", bufs=1, space="PSUM"))
            ps_db = phb.enter_context(tc.tile_pool(name="ps_db", bufs=2, space="PSUM"))

            def ln1_finish_apply(pend):
                p_acc_h, p_acc_sq, p_hp, p_ssl = pend
                mean_bc, rstd_bc = ln_finish(p_acc_h, p_acc_sq, float(E), SBB)
                for c in range(EC):
                    ln_apply_one(
                        nc.vector, p_hp[c], mean_bc, rstd_bc, g1_sb, be1_sb, c,
                        hlnt[:, c, p_ssl], SBB,
                    )

            pend = None
            for blk in range(s_total // SBB):
                ssl = slice(blk * SBB, (blk + 1) * SBB)
                exp_sb = pb.tile([P, NT, SBB], F8, tag="exp", name="exp", bufs=2)
                # --- scoresT -> exp (fp8, *S_E via ln-bias) ---
                for t in range(NT):
                    sp = ps_sc.tile([P, SBB], F32, tag="scps", name="scps")
                    for cp in range(EC // 2):
                        nc.tensor.matmul(
                            sp[:],
                            ktb[:, t, cp, :, :],
                            qtb[:, blk, cp, :, :],
                            start=(cp == 0),
                            stop=(cp == EC // 2 - 1),
                            perf_mode=DR,
                        )
                    nc.scalar.activation(
                        exp_sb[:, t, :], sp[:], AF.Exp,
                        scale=SCALE / (S_QK * S_QK), bias=lnse[:],
                    )
                # --- den = sum_t sum_p exp8 on the PE (ones vector) ---
                denp = ps_dn.tile([1, SBB], F32, tag="denp", name="denp")
                for t in range(NT):
                    nc.tensor.matmul(
                        denp[:],
                        ones8[:],
                        exp_sb[:, t, :],
                        start=(t == 0),
                        stop=(t == NT - 1),
                    )
                den_sb = den_pool.tile([1, SBB], F32, tag="den_sb", name="den_sb")
                nc.scalar.copy(den_sb[:], denp[:])
                # broadcast S_V*den to all partitions (fp32 matmul)
                denbc = ps_db.tile([P, SBB], F32, tag="denbc", name="denbc")
                nc.tensor.matmul(denbc[:], onesv[:], den_sb[:], start=True, stop=True)


                # --- attn_outT; h~ = S_V*den*x + av (no division: LN absorbs) ---
                acc_h = bc_pool.tile([P, SBB], F32, tag="acc_h", name="acc_h")
                acc_sq = bc_pool.tile([P, SBB], F32, tag="acc_sq", name="acc_sq")
                hp_tiles = []
                for et in range(EC):
                    avp = ps_av.tile([P, SBB], F32, tag="avps", name="avps")
                    for tp in range(NT // 2):
                        nc.tensor.matmul(
                            avp[:],
                            vb[:, tp, et, :, :],
                            exp_sb[:, 2 * tp : 2 * tp + 2, :],
                            start=(tp == 0),
                            stop=(tp == NT // 2 - 1),
                            perf_mode=DR,
                        )
                    t1 = tmp_pool.tile([P, SBB], F32, tag="resid", name="resid")
                    nc.vector.tensor_mul(t1[:], denbc[:], xtb_sb[:, et, ssl])
                    hp = hpre_pool.tile([P, SBB], BF16, tag="hpre", name="hpre")
                    nc.vector.tensor_add(hp[:], avp[:], t1[:])
                    hp_tiles.append(hp)
                    sq = sq_pool.tile([P, SBB], F32, tag="sq", name="sq")
                    nc.scalar.activation(sq[:], hp[:], AF.Square)
                    if et == 0:
                        nc.vector.tensor_copy(acc_h[:], hp[:])
                        nc.gpsimd.tensor_copy(acc_sq[:], sq[:])
                    else:
                        nc.vector.tensor_add(acc_h[:], acc_h[:], hp[:])
                        nc.gpsimd.tensor_add(acc_sq[:], acc_sq[:], sq[:])
                # previous block's LN1 tail is emitted after this block's
                # evictions: its DVE/ACT chain never head-of-line-blocks the
                # PSUM drain, and hpre (3 blocks deep) absorbs the deferral.
                if pend is not None:
                    ln1_finish_apply(pend)
                pend = (acc_h, acc_sq, hp_tiles, ssl)
            ln1_finish_apply(pend)

        # q/k/v fp8 tiles and xtb are dead after phase B — release their SBUF.
        stack_ab.close()

        # ================= Phase C: FFN + LN2 ==============================
        with ExitStack() as phc:
            pc = phc.enter_context(tc.tile_pool(name="pc", bufs=1))
            w1_pool = phc.enter_context(tc.tile_pool(name="w1p", bufs=3))
            w2_pool = phc.enter_context(tc.tile_pool(name="w2p", bufs=2))
            oev_pool = phc.enter_context(tc.tile_pool(name="oev", bufs=3))
            opre_pool = phc.enter_context(tc.tile_pool(name="opre", bufs=18))
            mm_ps = phc.enter_context(ExitStack())
            ps_f1 = mm_ps.enter_context(tc.tile_pool(name="ps_f1", bufs=4, space="PSUM"))
            ps_f2 = mm_ps.enter_context(tc.tile_pool(name="ps_f2", bufs=3, space="PSUM"))

            relu_sb = pc.tile([P, 2 * FC, SBC], BF16, tag="relu")
            NBC = s_total // SBC

            def ln2_finish_apply(pend):
                p_acc_h, p_acc_sq, p_op, p_ssl = pend
                mean2_bc, rstd2_bc = ln_finish(p_acc_h, p_acc_sq, float(E), SBC)
                for c in range(EC):
                    ot = oev_pool.tile([P, SBC], F32, tag="oev", name="oev")
                    ln_apply_one(nc.vector, p_op[c], mean2_bc, rstd2_bc, g2_sb, be2_sb, c, ot[:], SBC)
                    nc.sync.dma_start(aps["outt_o"][c * P : (c + 1) * P, p_ssl], ot[:])

            def ln2_finish_apply_pe(pend, ps_tail):
                """Last-block tail: stats reduce/broadcast on the PE, apply
                split across DVE and GpSimd — shortens the end-of-kernel
                serial chain (no gpsimd allreduce latency)."""
                p_acc_h, p_acc_sq, p_op, p_ssl = pend
                stp_h = ps_tail.tile([1, SBC], F32, tag="stph", name="stp_h", bufs=1)
                stp_q = ps_tail.tile([1, SBC], F32, tag="stpq", name="stp_q", bufs=1)
                nc.tensor.matmul(stp_h[:], onesr[:], p_acc_h[:], start=True, stop=True)
                nc.tensor.matmul(stp_q[:], onesr[:], p_acc_sq[:], start=True, stop=True)
                pk = den_pool2.tile([1, 2 * SBC], F32, tag="pk", name="pk")
                mean1 = pk[:, 0:SBC]
                rstd1 = pk[:, SBC : 2 * SBC]
                nc.vector.tensor_scalar_mul(mean1, stp_h[:], 1.0 / float(E))
                nc.vector.tensor_scalar_mul(rstd1, stp_q[:], 1.0 / float(E))
                m2 = den_pool2.tile([1, SBC], F32, tag="m2", name="m2")
                nc.vector.tensor_mul(m2[:], mean1, mean1)
                nc.vector.tensor_sub(rstd1, rstd1, m2[:])
                nc.scalar.activation(rstd1, rstd1, AF.Sqrt, bias=eps1[0:1, :])
                nc.vector.reciprocal(rstd1, rstd1)
                mean_bc = ps_tail.tile([P, SBC], F32, tag="mbc", name="mbc", bufs=1)
                rstd_bc = ps_tail.tile([P, SBC], F32, tag="rbc", name="rbc", bufs=1)
                nc.tensor.matmul(mean_bc[:], ones1[:], mean1, start=True, stop=True)
                nc.tensor.matmul(rstd_bc[:], ones1[:], rstd1, start=True, stop=True)
                mean_sb = bc_pool.tile([P, SBC], F32, tag="mean_bc", name="mean_sb", bufs=1)
                rstd_sb = bc_pool.tile([P, SBC], F32, tag="rstd_bc", name="rstd_sb", bufs=1)
                nc.scalar.copy(mean_sb[:], mean_bc[:])
                nc.scalar.copy(rstd_sb[:], rstd_bc[:])
                for c in range(EC):
                    eng = nc.vector if c < 6 else nc.gpsimd
                    ot = oev_pool.tile([P, SBC], F32, tag="oev", name="oev")
                    ln_apply_one(eng, p_op[c], mean_sb, rstd_sb, g2_sb, be2_sb, c, ot[:], SBC)
                    nc.sync.dma_start(aps["outt_o"][c * P : (c + 1) * P, p_ssl], ot[:])

            den_pool2 = phc.enter_context(tc.tile_pool(name="den2", bufs=1))

            def f1_group(blk, ssl, wt, fg):
                for fi in range(SBC // P):
                    ft = fg * (SBC // P) + fi
                    fp = ps_f1.tile([P, SBC], F32, tag="f1ps", name="f1ps")
                    for c in range(EC):
                        nc.tensor.matmul(
                            fp[:],
                            wt[:, c, fi * P : (fi + 1) * P],
                            hlnt[:, c, ssl],
                            start=(c == 0),
                            stop=(c == EC - 1),
                        )
                    nc.scalar.activation(
                        relu_sb[:, (blk % 2) * FC + ft, :],
                        fp[:], AF.Relu, bias=b1_sb[:, ft : ft + 1]
                    )

            def f2_one(blk, ssl, et, w2t, acc2_h, acc2_sq, op_tiles):
                op_ps = ps_f2.tile([P, SBC], F32, tag="f2ps", name="f2ps")
                for f in range(FC):
                    nc.tensor.matmul(
                        op_ps[:],
                        w2t[:, f, :],
                        relu_sb[:, (blk % 2) * FC + f, :],
                        start=(f == 0),
                        stop=(f == FC - 1),
                    )
                opc = opre_pool.tile([P, SBC], BF16, tag="opre", name="opre")
                nc.vector.scalar_tensor_tensor(
                    opc[:],
                    op_ps[:],
                    b2_sb[:, et : et + 1],
                    hlnt[:, et, ssl],
                    op0=OP.add,
                    op1=OP.add,
                )
                op_tiles.append(opc)
                sq = sq_pool.tile([P, SBC], F32, tag="sq", name="sq")
                nc.scalar.activation(sq[:], opc[:], AF.Square)
                if et == 0:
                    nc.vector.tensor_copy(acc2_h[:], opc[:])
                    nc.gpsimd.tensor_copy(acc2_sq[:], sq[:])
                else:
                    nc.vector.tensor_add(acc2_h[:], acc2_h[:], opc[:])
                    nc.gpsimd.tensor_add(acc2_sq[:], acc2_sq[:], sq[:])

            def load_w2(et):
                w2t = w2_pool.tile([P, FC, P], BF16, tag="w2t", name="w2t")
                nc.sync.dma_start(
                    w2t[:], aps["w2"][et].rearrange("p (f j) -> p f j", j=P)
                )
                return w2t

            # Blocks are processed in pairs sharing one W1/W2 streaming pass
            # (v4 re-streamed 16MB per block and saturated the DMA). The last
            # pair runs f2 block-sequential so only one block's LN2 remains
            # for the short PE-stats tail.
            def w1_tile(fg):
                wt = w1_pool.tile([P, EC, SBC], BF16, tag="w1", name="w1t")
                nc.sync.dma_start(
                    wt[:], aps["w1"][fg].rearrange("p (c j) -> p c j", c=EC)
                )
                return wt

            pends = []
            NPAIR = NBC // 2
            pend_last = None
            prefetched = []
            for bp in range(NPAIR):
                b0, b1 = 2 * bp, 2 * bp + 1
                ssl0 = slice(b0 * SBC, (b0 + 1) * SBC)
                ssl1 = slice(b1 * SBC, (b1 + 1) * SBC)
                for fg in range(F // SBC):
                    wt = prefetched[fg] if fg < len(prefetched) else w1_tile(fg)
                    f1_group(b0, ssl0, wt, fg)
                    f1_group(b1, ssl1, wt, fg)
                prefetched = []
                # pending LN2 tails run under the f1 window (DVE idle there)
                for pd in pends:
                    ln2_finish_apply(pd)
                pends = []
                if bp < NPAIR - 1:
                    acc_a = bc_pool.tile([P, SBC], F32, tag="acc_h", name="acc2_h")
                    acc_asq = bc_pool.tile([P, SBC], F32, tag="acc_sq", name="acc2_sq")
                    acc_b = bc_pool.tile([P, SBC], F32, tag="acc_h", name="acc2_h")
                    acc_bsq = bc_pool.tile([P, SBC], F32, tag="acc_sq", name="acc2_sq")
                    ops_a, ops_b = [], []
                    for et in range(EC):
                        w2t = load_w2(et)
                        f2_one(b0, ssl0, et, w2t, acc_a, acc_asq, ops_a)
                        f2_one(b1, ssl1, et, w2t, acc_b, acc_bsq, ops_b)
                        if et == 5:
                            # next pair's first W1 groups stream during this f2
                            prefetched = [w1_tile(g) for g in range(3)]
                    pends = [(acc_a, acc_asq, ops_a, ssl0), (acc_b, acc_bsq, ops_b, ssl1)]
                else:
                    acc_a = bc_pool.tile([P, SBC], F32, tag="acc_h", name="acc2_h")
                    acc_asq = bc_pool.tile([P, SBC], F32, tag="acc_sq", name="acc2_sq")
                    ops_a = []
                    for et in range(EC):
                        f2_one(b0, ssl0, et, load_w2(et), acc_a, acc_asq, ops_a)
                    # b0's LN2 tail: ps_f2's buffering absorbs the allreduce
                    # latency, so b1's evictions queue behind it harmlessly
                    ln2_finish_apply((acc_a, acc_asq, ops_a, ssl0))
                    acc_b = bc_pool.tile([P, SBC], F32, tag="acc_h", name="acc2_h")
                    acc_bsq = bc_pool.tile([P, SBC], F32, tag="acc_sq", name="acc2_sq")
                    ops_b = []
                    for et in range(EC):
                        f2_one(b1, ssl1, et, load_w2(et), acc_b, acc_bsq, ops_b)
                    pend_last = (acc_b, acc_bsq, ops_b, ssl1)
            mm_ps.close()
            with tc.tile_pool(name="ps_tail", bufs=1, space="PSUM") as ps_tail:
                ln2_finish_apply_pe(pend_last, ps_tail)


def _build(s_total, trivial):
    key = (s_total, trivial)
    if key in _BUILD_CACHE:
        return _BUILD_CACHE[key]
    nc = bacc.Bacc("TRN2", target_bir_lowering=False, debug=False, num_devices=B)
    aps = {}

    def din(name, shape, dt):
        aps[name] = nc.dram_tensor(name, shape, dt, kind="ExternalInput").ap()

    def dout(name, shape, dt):
        aps[name] = nc.dram_tensor(name, shape, dt, kind="ExternalOutput").ap()

    din("xtb", [s_total // SBA, P, EC * SBA], BF16)
    din("wq", [EC, P, E], BF16)        # [m][p][(c, j)] packed
    din("wk", [EC, P, E], BF16)
    din("wv", [P, EC * E], BF16)       # [p][(c, m, j)] packed
    din("w1", [F // SBC, P, EC * SBC], BF16)  # [fg][p][(c, j)] packed
    din("w2", [EC, P, F], BF16)        # [et][p][(f, j)] packed
    for nm, n in (("bq", E), ("bk", E), ("bv", E), ("b1", F), ("b2", E),
                  ("gamma1", E), ("beta1", E), ("gamma2", E), ("beta2", E)):
        din(nm, [n], F32)
    dout("qt_o", [E, s_total], F32)
    dout("kt_o", [E, s_total], F32)
    dout("v_o", [s_total, E], F32)
    dout("outt_o", [E, s_total], F32)

    with tile.TileContext(nc) as tc:
        _emit(tc, aps, s_total, trivial)
    nc.compile()
    _BUILD_CACHE[key] = (nc, aps)
    return nc, aps


def _pack_qk(w):
    # out[m, p, c*128+j] = w[c*128+p, m*128+j]
    return np.ascontiguousarray(
        w.reshape(EC, P, EC, P).transpose(2, 1, 0, 3).reshape(EC, P, E)
    )


def _pack_wv(w):
    # out[p, (c, m, j)] = w[c*128+p, m*128+j]
    return np.ascontiguousarray(
        w.reshape(EC, P, EC, P).transpose(1, 0, 2, 3).reshape(P, EC * E)
    )


def _pack_w1(w):
    # out[fg, p, c*512+j] = w1[c*128+p, fg*512+j]
    return np.ascontiguousarray(
        w.reshape(EC, P, F // SBC, SBC).transpose(2, 1, 0, 3).reshape(F // SBC, P, EC * SBC)
    )


def _pack_w2(w):
    # out[et, p, (f, j)] = w[f*128+p, et*128+j]
    return np.ascontiguousarray(
        w.reshape(FC, P, EC, P).transpose(2, 1, 0, 3).reshape(EC, P, F)
    )


def _prep_in_maps(inputs):
    bf = ml_dtypes.bfloat16
    x = np.ascontiguousarray(np.asarray(inputs["x"], dtype=np.float32))
    shared = {
        "wq": _pack_qk(np.asarray(inputs["Wq"], np.float32).astype(bf)),
        "wk": _pack_qk(np.asarray(inputs["Wk"], np.float32).astype(bf)),
        "wv": _pack_wv(np.asarray(inputs["Wv"], np.float32).astype(bf)),
        "w1": _pack_w1(np.asarray(inputs["W1"], np.float32).astype(bf)),
        "w2": _pack_w2(np.asarray(inputs["W2"], np.float32).astype(bf)),
    }
    for nm in ("bq", "bk", "bv", "b1", "b2", "gamma1", "beta1", "gamma2", "beta2"):
        shared[nm] = np.asarray(inputs[nm], np.float32)
    in_maps = []
    for b in range(x.shape[0]):
        m = dict(shared)
        xt = x[b].T.astype(bf)  # [E, S]
        m["xtb"] = np.ascontiguousarray(
            xt.reshape(EC, P, S // SBA, SBA).transpose(2, 1, 0, 3).reshape(S // SBA, P, EC * SBA)
        )
        in_maps.append(m)
    return in_maps


def kernel(**inputs):
    trivial = all(
        not np.any(np.asarray(inputs[k])) for k in ("bv", "beta1", "beta2")
    ) and all(np.all(np.asarray(inputs[k]) == 1.0) for k in ("gamma1", "gamma2"))
    nc, _ = _build(S, trivial)
    in_maps = _prep_in_maps(inputs)
    res = bass_utils.run_bass_kernel_spmd(nc, in_maps, core_ids=list(range(B)))
    out = np.empty((B, S, E), np.float32)
    q = np.empty((B, S, E), np.float32)
    k = np.empty((B, S, E), np.float32)
    v = np.empty((B, S, E), np.float32)
    for b in range(B):
        r = res.results[b]
        out[b] = r["outt_o"].T
        q[b] = r["qt_o"].T
        k[b] = r["kt_o"].T
        v[b] = r["v_o"]
    return (out, k, q, v)


# revision 20
# speedup vs baseline: 1.1325x; 1.0674x over previous
"""Trainium2 Bass kernel for a BERT encoder block (single-head attention + FFN).

Sharding: data-parallel over batch — B=8 batches across 8 NeuronCores, one
batch element per core. No collectives.

v2: fp8 (e4m3) DoubleRow attention. q/k/v are projected in bf16 (their fp32
results are kernel outputs), then requantized to scaled fp8. Scores and
attn*v contract two 128-k-tiles per PE instruction (DoubleRow) at 2x bf16
throughput. The softmax denominator is reduced on the PE with an fp8 ones
vector and never divides anything: LayerNorm is invariant to per-column
scaling, so phase B forms h~ = (S_V*den)*x + attn_psum and LN1 absorbs den.
FFN stays bf16 (fp8 would breach the error budget). All matmuls stream
N=512 moving columns (the v projection streamed N=128 in v1).
"""

import sys

if "/opt/trn_rl_repo" not in sys.path:
    sys.path.insert(0, "/opt/trn_rl_repo")

from contextlib import ExitStack

import ml_dtypes
import numpy as np

import concourse.bass as bass
import concourse.tile as tile
from concourse import bacc, bass_utils, mybir
from concourse.bass_isa import ReduceOp

F32 = mybir.dt.float32
BF16 = mybir.dt.bfloat16
F8 = mybir.dt.float8e4
AF = mybir.ActivationFunctionType
OP = mybir.AluOpType
DR = mybir.MatmulPerfMode.DoubleRow

B = 8
S = 2048
E = 1024
F = 4096
P = 128
SBA = 512  # seq block width, qkv projection phase
SBB = 512  # seq block width, attention + LN1 phase
SBC = 512  # seq block width, FFN + LN2 phase
EC = E // P  # 8
FC = F // P  # 32
EPS = 1e-5
SCALE = 1.0 / float(np.sqrt(E))
S_QK = 16.0  # fp8 scale for q/k tiles
S_E = 8.0    # fp8 scale for exp tiles
S_V = 16.0   # fp8 scale for v tiles

_BUILD_CACHE = {}


def _emit(tc, aps, s_total, trivial):
    nc = tc.nc
    NT = s_total // P

    with ExitStack() as outer:
        # ---- constants & small shared pools -------------------------------
        consts = outer.enter_context(tc.tile_pool(name="consts", bufs=1))
        eps1 = consts.tile([P, 1], F32, tag="eps1")
        nc.vector.memset(eps1[:], EPS)
        lnse = consts.tile([P, 1], F32, tag="lnse")
        nc.vector.memset(lnse[:], float(np.log(S_E)))
        ones8 = consts.tile([P, 1], F8, tag="ones8")
        nc.vector.memset(ones8[:], 1.0)
        onesv = consts.tile([1, P], F32, tag="onesv")
        nc.vector.memset(onesv[:], S_V)
        ones1 = consts.tile([1, P], F32, tag="ones1")
        nc.vector.memset(ones1[:], 1.0)
        onesr = consts.tile([P, 1], F32, tag="onesr")
        nc.vector.memset(onesr[:], 1.0)

        def chunked_bias(name, src_ap, nchunk):
            t = consts.tile([P, nchunk], F32, tag=name, name=name)
            nc.sync.dma_start(t[:], src_ap.rearrange("(c p) -> p c", p=P))
            return t

        bq_sb = chunked_bias("bq", aps["bq"], EC)
        bk_sb = chunked_bias("bk", aps["bk"], EC)
        b1_sb = chunked_bias("b1", aps["b1"], FC)
        b2_sb = chunked_bias("b2", aps["b2"], EC)
        g1_sb = chunked_bias("g1", aps["gamma1"], EC)
        g2_sb = chunked_bias("g2", aps["gamma2"], EC)
        # scaled copies of bq/bk for the fp8 requantized q/k tiles
        bq16 = consts.tile([P, EC], F32, tag="bq16")
        nc.vector.tensor_scalar_mul(bq16[:], bq_sb[:], S_QK)
        bk16 = consts.tile([P, EC], F32, tag="bk16")
        nc.vector.tensor_scalar_mul(bk16[:], bk_sb[:], S_QK)
        be1_sb = be2_sb = None
        bv_bc = consts.tile([P, E], F32, tag="bv_bc")
        if not trivial:
            be1_sb = chunked_bias("be1", aps["beta1"], EC)
            be2_sb = chunked_bias("be2", aps["beta2"], EC)
            bv_ap = aps["bv"]
            nc.sync.dma_start(
                bv_bc[:],
                bass.AP(tensor=bv_ap.tensor, offset=bv_ap.offset, ap=[[0, P]] + list(bv_ap.ap)),
            )
        else:
            nc.vector.memset(bv_bc[:], 0.0)

        # `late` pools are opened at phase-B start (so phase A has the SBUF)
        # but released only at the very end (they serve phases B and C).
        late = outer.enter_context(ExitStack())

        stack_ab = outer.enter_context(ExitStack())
        ab = stack_ab.enter_context(tc.tile_pool(name="ab", bufs=1))
        # fp8 operand layouts keep each DoubleRow k-tile pair contiguous
        # (s3_lw_dual_fp8 ISA restriction): [.., pair-idx, 2, tail].
        NBB = s_total // SBB
        qtb = ab.tile([P, NBB, EC // 2, 2, SBB], F8, tag="qtb")
        ktb = ab.tile([P, NT, EC // 2, 2, P], F8, tag="ktb")
        vb = ab.tile([P, NT // 2, EC, 2, P], F8, tag="vb")
        xtb_sb = ab.tile([P, EC, s_total], BF16, tag="xtb")

        # ================= Phase A: q/k/v projections ======================
        with ExitStack() as pha:
            evf = pha.enter_context(tc.tile_pool(name="evf", bufs=6))

            NBA = s_total // SBA
            # blocked host layout: one 1MB DMA per seq block with 8KB
            # contiguous lines (strided 1KB lines gated kernel start in v2)
            for pz in range(NBA):
                zsl = slice(pz * SBA, (pz + 1) * SBA)
                nc.sync.dma_start(
                    xtb_sb[:, :, zsl],
                    aps["xtb"][pz].rearrange("p (c s) -> p c s", c=EC),
                )

            # --- qT and kT: packed weights [m][p][c][j] streamed per m ---
            with tc.tile_pool(name="wqk", bufs=3) as wqk_pool, \
                 tc.tile_pool(name="ps_qk", bufs=8, space="PSUM") as ps_qk:
                for w_ap, b_sb, bsc, o_ap, tb in (
                    (aps["wq"], bq_sb, bq16, aps["qt_o"], qtb),
                    (aps["wk"], bk_sb, bk16, aps["kt_o"], ktb),
                ):
                    for m in range(EC):
                        wt = wqk_pool.tile([P, EC, P], BF16, tag="wqk", name="wqk")
                        nc.sync.dma_start(
                            wt[:], w_ap[m].rearrange("p (c j) -> p c j", j=P)
                        )
                        pts = []
                        for blk in range(NBA):
                            pts.append(ps_qk.tile([P, SBA], F32, tag="projps", name="projps"))
                        for c in range(EC):
                            for blk in range(NBA):
                                nc.tensor.matmul(
                                    pts[blk][:],
                                    wt[:, c, :],
                                    xtb_sb[:, c, blk * SBA : (blk + 1) * SBA],
                                    start=(c == 0),
                                    stop=(c == EC - 1),
                                )
                        for blk in range(NBA):
                            f32t = evf.tile([P, SBA], F32, tag="evf", name="evf")
                            nc.vector.tensor_scalar_add(f32t[:], pts[blk][:], b_sb[:, m : m + 1])
                            if tb is qtb:
                                f8dst = qtb[:, blk, m // 2, m % 2, :]
                            else:
                                f8dst = ktb[:, 4 * blk : 4 * (blk + 1), m // 2, m % 2, :]
                            nc.scalar.activation(
                                f8dst,
                                pts[blk][:],
                                AF.Identity,
                                bias=bsc[:, m : m + 1],
                                scale=S_QK,
                            )
                            nc.sync.dma_start(
                                o_ap[m * P : (m + 1) * P, blk * SBA : (blk + 1) * SBA], f32t[:]
                            )

            # wv packed [p][(c, m, j)]: moving slices of 512 e-columns
            wv_pool = pha.enter_context(tc.tile_pool(name="wvp", bufs=1))
            wv_sb = wv_pool.tile([P, EC, E], BF16, tag="wv")
            nc.sync.dma_start(
                wv_sb[:], aps["wv"].rearrange("p (c n) -> p c n", c=EC)
            )

            # --- v natural: xT stationary, wv moving with N=512 ---
            with tc.tile_pool(name="ps_v", bufs=3, space="PSUM", side="right") as ps_v:
                for st in range(NT):
                    for eb in range(E // SBA):
                        esl = slice(eb * SBA, (eb + 1) * SBA)
                        vp = ps_v.tile([P, SBA], F32, tag="vps", name="vps")
                        for c in range(EC):
                            nc.tensor.matmul(
                                vp[:],
                                xtb_sb[:, c, st * P : (st + 1) * P],
                                wv_sb[:, c, esl],
                                start=(c == 0),
                                stop=(c == EC - 1),
                            )
                        f32t = evf.tile([P, SBA], F32, tag="evf", name="evf")
                        nc.vector.tensor_add(f32t[:], vp[:], bv_bc[:, esl])
                        nc.scalar.activation(
                            vb[:, st // 2, 4 * eb : 4 * (eb + 1), st % 2, :],
                            f32t[:], AF.Identity, scale=S_V,
                        )
                        nc.sync.dma_start(aps["v_o"][st * P : (st + 1) * P, esl], f32t[:])

        # ================= Phase B: attention + LN1 ========================
        hln = late.enter_context(tc.tile_pool(name="hln", bufs=1, side="right"))
        hlnt = hln.tile([P, EC, s_total], BF16, tag="hlnt")
        bc_pool = late.enter_context(tc.tile_pool(name="bc", bufs=2, side="right"))
        tmp_pool = late.enter_context(tc.tile_pool(name="tmp", bufs=2, side="right"))
        sq_pool = late.enter_context(tc.tile_pool(name="sq", bufs=2, side="right"))

        def ln_finish_post(acc_h, acc_sq, n, w):
            """mean/rstd from already-allreduced sums; DVE+ACT only."""
            mean_bc = bc_pool.tile([P, w], F32, tag="mean_bc", name="mean_bc", bufs=1)
            nc.vector.tensor_scalar_mul(mean_bc[:], acc_h[:], 1.0 / n)
            var_bc = bc_pool.tile([P, w], F32, tag="var_bc", name="var_bc", bufs=1)
            nc.vector.tensor_scalar_mul(var_bc[:], acc_sq[:], 1.0 / n)
            m2 = tmp_pool.tile([P, w], F32, tag="lnsub", name="m2")
            nc.vector.tensor_mul(m2[:], mean_bc[:], mean_bc[:])
            nc.vector.tensor_sub(var_bc[:], var_bc[:], m2[:])
            nc.scalar.activation(var_bc[:], var_bc[:], AF.Sqrt, bias=eps1[:])
            rstd_bc = bc_pool.tile([P, w], F32, tag="rstd_bc", name="rstd_bc", bufs=1)
            nc.vector.reciprocal(rstd_bc[:], var_bc[:])
            return mean_bc, rstd_bc

        def ln_apply_one(eng, src, mean_bc, rstd_bc, g_sb, be_sb2, c, dst, w):
            t1 = tmp_pool.tile([P, w], F32, tag="lnsub", name="lnsub")
            eng.tensor_sub(t1[:], src[:], mean_bc[:])
            if trivial:
                eng.tensor_mul(dst, t1[:], rstd_bc[:])
            else:
                eng.scalar_tensor_tensor(
                    dst, t1[:], g_sb[:, c : c + 1], rstd_bc[:], op0=OP.mult, op1=OP.mult
                )
                eng.tensor_scalar_add(dst, dst, be_sb2[:, c : c + 1])

        with ExitStack() as phb:
            pb = phb.enter_context(tc.tile_pool(name="pb", bufs=1))
            hpre_pool = phb.enter_context(tc.tile_pool(name="hpre", bufs=24))
            den_pool = phb.enter_context(tc.tile_pool(name="den", bufs=2))
            ps_sc = phb.enter_context(tc.tile_pool(name="ps_sc", bufs=2, space="PSUM"))
            ps_av = phb.enter_context(tc.tile_pool(name="ps_av", bufs=3, space="PSUM"))
            ps_dn = phb.enter_context(tc.tile_pool(name="ps_dn", bufs=1, space="PSUM"))
            ps_db = phb.enter_context(tc.tile_pool(name="ps_db", bufs=2, space="PSUM"))

            def ln1_reduce(pend):
                # gpsimd allreduce(s) emitted as soon as the accs are complete
                p_acc_h, p_acc_sq, p_hp, p_ssl = pend
                nc.gpsimd.partition_all_reduce(p_acc_h[:], p_acc_h[:], P, ReduceOp.add)
                if not trivial:
                    nc.gpsimd.partition_all_reduce(p_acc_sq[:], p_acc_sq[:], P, ReduceOp.add)

            def ln1_finish_apply(pend):
                p_acc_h, p_acc_sq, p_hp, p_ssl = pend
                if trivial:
                    # sigma-free LN1: beta1=0 and b1=b2=0, relu is positively
                    # homogeneous and LN2 is per-column scale-invariant, so the
                    # 1/sigma factor is dropped and LN2 absorbs it. hlnt is the
                    # mean-centered h~ (column scale ~S_V*den).
                    mean_bc = bc_pool.tile([P, SBB], F32, tag="mean_bc", name="mean_bc", bufs=1)
                    nc.vector.tensor_scalar_mul(mean_bc[:], p_acc_h[:], 1.0 / float(E))
                    for c in range(EC):
                        nc.vector.tensor_sub(hlnt[:, c, p_ssl], p_hp[c][:], mean_bc[:])
                    return
                mean_bc, rstd_bc = ln_finish_post(p_acc_h, p_acc_sq, float(E), SBB)
                for c in range(EC):
                    ln_apply_one(
                        nc.vector, p_hp[c], mean_bc, rstd_bc, g1_sb, be1_sb, c,
                        hlnt[:, c, p_ssl], SBB,
                    )

            pend = None
            for blk in range(s_total // SBB):
                ssl = slice(blk * SBB, (blk + 1) * SBB)
                exp_sb = pb.tile([P, NT, SBB], F8, tag="exp", name="exp", bufs=2)
                # --- scoresT -> exp (fp8, *S_E via ln-bias) ---
                for t in range(NT):
                    sp = ps_sc.tile([P, SBB], F32, tag="scps", name="scps")
                    for cp in range(EC // 2):
                        nc.tensor.matmul(
                            sp[:],
                            ktb[:, t, cp, :, :],
                            qtb[:, blk, cp, :, :],
                            start=(cp == 0),
                            stop=(cp == EC // 2 - 1),
                            perf_mode=DR,
                        )
                    nc.scalar.activation(
                        exp_sb[:, t, :], sp[:], AF.Exp,
                        scale=SCALE / (S_QK * S_QK), bias=lnse[:],
                    )
                # --- den = sum_t sum_p exp8 on the PE (ones vector) ---
                denp = ps_dn.tile([1, SBB], F32, tag="denp", name="denp")
                for t in range(NT):
                    nc.tensor.matmul(
                        denp[:],
                        ones8[:],
                        exp_sb[:, t, :],
                        start=(t == 0),
                        stop=(t == NT - 1),
                    )
                den_sb = den_pool.tile([1, SBB], F32, tag="den_sb", name="den_sb")
                nc.scalar.copy(den_sb[:], denp[:])
                # broadcast S_V*den to all partitions (fp32 matmul)
                denbc = ps_db.tile([P, SBB], F32, tag="denbc", name="denbc")
                nc.tensor.matmul(denbc[:], onesv[:], den_sb[:], start=True, stop=True)


                # --- attn_outT; h~ = S_V*den*x + av (no division: LN absorbs) ---
                acc_h = bc_pool.tile([P, SBB], F32, tag="acc_h", name="acc_h")
                acc_sq = None
                if not trivial:
                    acc_sq = bc_pool.tile([P, SBB], F32, tag="acc_sq", name="acc_sq")
                hp_tiles = []
                for et in range(EC):
                    avp = ps_av.tile([P, SBB], F32, tag="avps", name="avps")
                    for tp in range(NT // 2):
                        nc.tensor.matmul(
                            avp[:],
                            vb[:, tp, et, :, :],
                            exp_sb[:, 2 * tp : 2 * tp + 2, :],
                            start=(tp == 0),
                            stop=(tp == NT // 2 - 1),
                            perf_mode=DR,
                        )
                    t1 = tmp_pool.tile([P, SBB], F32, tag="resid", name="resid")
                    nc.vector.tensor_mul(t1[:], denbc[:], xtb_sb[:, et, ssl])
                    hp = hpre_pool.tile([P, SBB], BF16, tag="hpre", name="hpre")
                    nc.vector.tensor_add(hp[:], avp[:], t1[:])
                    hp_tiles.append(hp)
                    if trivial:
                        if et == 0:
                            nc.gpsimd.tensor_copy(acc_h[:], hp[:])
                        else:
                            nc.gpsimd.tensor_add(acc_h[:], acc_h[:], hp[:])
                    else:
                        sq = sq_pool.tile([P, SBB], F32, tag="sq", name="sq")
                        nc.scalar.activation(sq[:], hp[:], AF.Square)
                        if et == 0:
                            nc.vector.tensor_copy(acc_h[:], hp[:])
                            nc.gpsimd.tensor_copy(acc_sq[:], sq[:])
                        else:
                            nc.vector.tensor_add(acc_h[:], acc_h[:], hp[:])
                            nc.gpsimd.tensor_add(acc_sq[:], acc_sq[:], sq[:])
                # previous block's LN1 tail: DVE chain after this block's
                # evictions (never blocks the PSUM drain); its allreduce was
                # already emitted during this block's scores.
                if pend is not None:
                    ln1_finish_apply(pend)
                pend = (acc_h, acc_sq, hp_tiles, ssl)
                ln1_reduce(pend)
            ln1_finish_apply(pend)

        # q/k/v fp8 tiles and xtb are dead after phase B — release their SBUF.
        stack_ab.close()

        # ================= Phase C: FFN + LN2 ==============================
        with ExitStack() as phc:
            pc = phc.enter_context(tc.tile_pool(name="pc", bufs=1))
            w1_pool = phc.enter_context(tc.tile_pool(name="w1p", bufs=3))
            w2_pool = phc.enter_context(tc.tile_pool(name="w2p", bufs=2))
            oev_pool = phc.enter_context(tc.tile_pool(name="oev", bufs=3))
            opre_pool = phc.enter_context(tc.tile_pool(name="opre", bufs=18))
            mm_ps = phc.enter_context(ExitStack())
            ps_f1 = mm_ps.enter_context(tc.tile_pool(name="ps_f1", bufs=4, space="PSUM"))
            ps_f2 = mm_ps.enter_context(tc.tile_pool(name="ps_f2", bufs=3, space="PSUM"))

            relu_sb = pc.tile([P, 2 * FC, SBC], BF16, tag="relu")
            NBC = s_total // SBC

            def ln2_reduce(pend):
                p_acc_h, p_acc_sq, p_op, p_ssl = pend
                nc.gpsimd.partition_all_reduce(p_acc_h[:], p_acc_h[:], P, ReduceOp.add)
                nc.gpsimd.partition_all_reduce(p_acc_sq[:], p_acc_sq[:], P, ReduceOp.add)

            def ln2_finish_apply(pend):
                p_acc_h, p_acc_sq, p_op, p_ssl = pend
                mean2_bc, rstd2_bc = ln_finish_post(p_acc_h, p_acc_sq, float(E), SBC)
                for c in range(EC):
                    ot = oev_pool.tile([P, SBC], F32, tag="oev", name="oev")
                    ln_apply_one(nc.vector, p_op[c], mean2_bc, rstd2_bc, g2_sb, be2_sb, c, ot[:], SBC)
                    nc.sync.dma_start(aps["outt_o"][c * P : (c + 1) * P, p_ssl], ot[:])

            def ln2_finish_apply_pe(pend, ps_tail):
                """Last-block tail: stats reduce/broadcast on the PE, apply
                split across DVE and GpSimd — shortens the end-of-kernel
                serial chain (no gpsimd allreduce latency)."""
                p_acc_h, p_acc_sq, p_op, p_ssl = pend
                stp_h = ps_tail.tile([1, SBC], F32, tag="stph", name="stp_h", bufs=1)
                stp_q = ps_tail.tile([1, SBC], F32, tag="stpq", name="stp_q", bufs=1)
                nc.tensor.matmul(stp_h[:], onesr[:], p_acc_h[:], start=True, stop=True)
                nc.tensor.matmul(stp_q[:], onesr[:], p_acc_sq[:], start=True, stop=True)
                pk = den_pool2.tile([1, 2 * SBC], F32, tag="pk", name="pk")
                mean1 = pk[:, 0:SBC]
                rstd1 = pk[:, SBC : 2 * SBC]
                nc.vector.tensor_scalar_mul(mean1, stp_h[:], 1.0 / float(E))
                nc.vector.tensor_scalar_mul(rstd1, stp_q[:], 1.0 / float(E))
                m2 = den_pool2.tile([1, SBC], F32, tag="m2", name="m2")
                nc.vector.tensor_mul(m2[:], mean1, mean1)
                nc.vector.tensor_sub(rstd1, rstd1, m2[:])
                nc.scalar.activation(rstd1, rstd1, AF.Sqrt, bias=eps1[0:1, :])
                nc.vector.reciprocal(rstd1, rstd1)
                mean_bc = ps_tail.tile([P, SBC], F32, tag="mbc", name="mbc", bufs=1)
                rstd_bc = ps_tail.tile([P, SBC], F32, tag="rbc", name="rbc", bufs=1)
                nc.tensor.matmul(mean_bc[:], ones1[:], mean1, start=True, stop=True)
                nc.tensor.matmul(rstd_bc[:], ones1[:], rstd1, start=True, stop=True)
                mean_sb = bc_pool.tile([P, SBC], F32, tag="mean_bc", name="mean_sb", bufs=1)
                rstd_sb = bc_pool.tile([P, SBC], F32, tag="rstd_bc", name="rstd_sb", bufs=1)
                nc.scalar.copy(mean_sb[:], mean_bc[:])
                nc.scalar.copy(rstd_sb[:], rstd_bc[:])
                for c in range(EC):
                    eng = nc.vector if c < 6 else nc.gpsimd
                    ot = oev_pool.tile([P, SBC], F32, tag="oev", name="oev")
                    ln_apply_one(eng, p_op[c], mean_sb, rstd_sb, g2_sb, be2_sb, c, ot[:], SBC)
                    nc.sync.dma_start(aps["outt_o"][c * P : (c + 1) * P, p_ssl], ot[:])

            den_pool2 = phc.enter_context(tc.tile_pool(name="den2", bufs=1))

            def f1_group(blk, ssl, wt, fg):
                for fi in range(SBC // P):
                    ft = fg * (SBC // P) + fi
                    fp = ps_f1.tile([P, SBC], F32, tag="f1ps", name="f1ps")
                    for c in range(EC):
                        nc.tensor.matmul(
                            fp[:],
                            wt[:, c, fi * P : (fi + 1) * P],
                            hlnt[:, c, ssl],
                            start=(c == 0),
                            stop=(c == EC - 1),
                        )
                    nc.scalar.activation(
                        relu_sb[:, (blk % 2) * FC + ft, :],
                        fp[:], AF.Relu, bias=b1_sb[:, ft : ft + 1]
                    )

            def f2_one(blk, ssl, et, w2t, acc2_h, acc2_sq, op_tiles):
                op_ps = ps_f2.tile([P, SBC], F32, tag="f2ps", name="f2ps")
                for f in range(FC):
                    nc.tensor.matmul(
                        op_ps[:],
                        w2t[:, f, :],
                        relu_sb[:, (blk % 2) * FC + f, :],
                        start=(f == 0),
                        stop=(f == FC - 1),
                    )
                opc = opre_pool.tile([P, SBC], BF16, tag="opre", name="opre")
                nc.vector.scalar_tensor_tensor(
                    opc[:],
                    op_ps[:],
                    b2_sb[:, et : et + 1],
                    hlnt[:, et, ssl],
                    op0=OP.add,
                    op1=OP.add,
                )
                op_tiles.append(opc)
                sq = sq_pool.tile([P, SBC], F32, tag="sq", name="sq")
                nc.vector.tensor_mul(sq[:], opc[:], opc[:])
                if et == 0:
                    nc.gpsimd.tensor_copy(acc2_h[:], opc[:])
                    nc.vector.tensor_copy(acc2_sq[:], sq[:])
                else:
                    nc.gpsimd.tensor_add(acc2_h[:], acc2_h[:], opc[:])
                    nc.vector.tensor_add(acc2_sq[:], acc2_sq[:], sq[:])

            def load_w2(et):
                w2t = w2_pool.tile([P, FC, P], BF16, tag="w2t", name="w2t")
                nc.sync.dma_start(
                    w2t[:], aps["w2"][et].rearrange("p (f j) -> p f j", j=P)
                )
                return w2t

            # Blocks are processed in pairs sharing one W1/W2 streaming pass
            # (v4 re-streamed 16MB per block and saturated the DMA). The last
            # pair runs f2 block-sequential so only one block's LN2 remains
            # for the short PE-stats tail.
            def w1_tile(fg):
                wt = w1_pool.tile([P, EC, SBC], BF16, tag="w1", name="w1t")
                nc.sync.dma_start(
                    wt[:], aps["w1"][fg].rearrange("p (c j) -> p c j", c=EC)
                )
                return wt

            pends = []
            NPAIR = NBC // 2
            pend_last = None
            prefetched = []
            for bp in range(NPAIR):
                b0, b1 = 2 * bp, 2 * bp + 1
                ssl0 = slice(b0 * SBC, (b0 + 1) * SBC)
                ssl1 = slice(b1 * SBC, (b1 + 1) * SBC)
                for fg in range(F // SBC):
                    wt = prefetched[fg] if fg < len(prefetched) else w1_tile(fg)
                    f1_group(b0, ssl0, wt, fg)
                    f1_group(b1, ssl1, wt, fg)
                prefetched = []
                # pending LN2 tails run under the f1 window (DVE idle there)
                for pd in pends:
                    ln2_finish_apply(pd)
                pends = []
                if bp < NPAIR - 1:
                    acc_a = bc_pool.tile([P, SBC], F32, tag="acc_h", name="acc2_h")
                    acc_asq = bc_pool.tile([P, SBC], F32, tag="acc_sq", name="acc2_sq")
                    acc_b = bc_pool.tile([P, SBC], F32, tag="acc_h", name="acc2_h")
                    acc_bsq = bc_pool.tile([P, SBC], F32, tag="acc_sq", name="acc2_sq")
                    ops_a, ops_b = [], []
                    for et in range(EC):
                        w2t = load_w2(et)
                        f2_one(b0, ssl0, et, w2t, acc_a, acc_asq, ops_a)
                        f2_one(b1, ssl1, et, w2t, acc_b, acc_bsq, ops_b)
                        if et == 5:
                            # next pair's first W1 groups stream during this f2
                            prefetched = [w1_tile(g) for g in range(3)]
                    pends = [(acc_a, acc_asq, ops_a, ssl0), (acc_b, acc_bsq, ops_b, ssl1)]
                    for pd in pends:
                        ln2_reduce(pd)
                else:
                    acc_a = bc_pool.tile([P, SBC], F32, tag="acc_h", name="acc2_h")
                    acc_asq = bc_pool.tile([P, SBC], F32, tag="acc_sq", name="acc2_sq")
                    ops_a = []
                    for et in range(EC):
                        f2_one(b0, ssl0, et, load_w2(et), acc_a, acc_asq, ops_a)
                    # b0's LN2 tail: allreduce first (data-ready), then the
                    # short DVE chain; ps_f2's buffering absorbs the latency
                    ln2_reduce((acc_a, acc_asq, ops_a, ssl0))
                    ln2_finish_apply((acc_a, acc_asq, ops_a, ssl0))
                    acc_b = bc_pool.tile([P, SBC], F32, tag="acc_h", name="acc2_h")
                    acc_bsq = bc_pool.tile([P, SBC], F32, tag="acc_sq", name="acc2_sq")
                    ops_b = []
                    for et in range(EC):
                        f2_one(b1, ssl1, et, load_w2(et), acc_b, acc_bsq, ops_b)
                    pend_last = (acc_b, acc_bsq, ops_b, ssl1)
            mm_ps.close()
            with tc.tile_pool(name="ps_tail", bufs=1, space="PSUM") as ps_tail:
                ln2_finish_apply_pe(pend_last, ps_tail)


def _build(s_total, trivial):
    key = (s_total, trivial)
    if key in _BUILD_CACHE:
        return _BUILD_CACHE[key]
    nc = bacc.Bacc("TRN2", target_bir_lowering=False, debug=False, num_devices=B)
    aps = {}

    def din(name, shape, dt):
        aps[name] = nc.dram_tensor(name, shape, dt, kind="ExternalInput").ap()

    def dout(name, shape, dt):
        aps[name] = nc.dram_tensor(name, shape, dt, kind="ExternalOutput").ap()

    din("xtb", [s_total // SBA, P, EC * SBA], BF16)
    din("wq", [EC, P, E], BF16)        # [m][p][(c, j)] packed
    din("wk", [EC, P, E], BF16)
    din("wv", [P, EC * E], BF16)       # [p][(c, m, j)] packed
    din("w1", [F // SBC, P, EC * SBC], BF16)  # [fg][p][(c, j)] packed
    din("w2", [EC, P, F], BF16)        # [et][p][(f, j)] packed
    for nm, n in (("bq", E), ("bk", E), ("bv", E), ("b1", F), ("b2", E),
                  ("gamma1", E), ("beta1", E), ("gamma2", E), ("beta2", E)):
        din(nm, [n], F32)
    dout("qt_o", [E, s_total], F32)
    dout("kt_o", [E, s_total], F32)
    dout("v_o", [s_total, E], F32)
    dout("outt_o", [E, s_total], F32)

    with tile.TileContext(nc) as tc:
        _emit(tc, aps, s_total, trivial)
    nc.compile()
    _BUILD_CACHE[key] = (nc, aps)
    return nc, aps


def _pack_qk(w):
    # out[m, p, c*128+j] = w[c*128+p, m*128+j]
    return np.ascontiguousarray(
        w.reshape(EC, P, EC, P).transpose(2, 1, 0, 3).reshape(EC, P, E)
    )


def _pack_wv(w):
    # out[p, (c, m, j)] = w[c*128+p, m*128+j]
    return np.ascontiguousarray(
        w.reshape(EC, P, EC, P).transpose(1, 0, 2, 3).reshape(P, EC * E)
    )


def _pack_w1(w):
    # out[fg, p, c*512+j] = w1[c*128+p, fg*512+j]
    return np.ascontiguousarray(
        w.reshape(EC, P, F // SBC, SBC).transpose(2, 1, 0, 3).reshape(F // SBC, P, EC * SBC)
    )


def _pack_w2(w):
    # out[et, p, (f, j)] = w[f*128+p, et*128+j]
    return np.ascontiguousarray(
        w.reshape(FC, P, EC, P).transpose(2, 1, 0, 3).reshape(EC, P, F)
    )


def _prep_in_maps(inputs):
    bf = ml_dtypes.bfloat16
    x = np.ascontiguousarray(np.asarray(inputs["x"], dtype=np.float32))
    shared = {
        "wq": _pack_qk(np.asarray(inputs["Wq"], np.float32).astype(bf)),
        "wk": _pack_qk(np.asarray(inputs["Wk"], np.float32).astype(bf)),
        "wv": _pack_wv(np.asarray(inputs["Wv"], np.float32).astype(bf)),
        "w1": _pack_w1(np.asarray(inputs["W1"], np.float32).astype(bf)),
        "w2": _pack_w2(np.asarray(inputs["W2"], np.float32).astype(bf)),
    }
    for nm in ("bq", "bk", "bv", "b1", "b2", "gamma1", "beta1", "gamma2", "beta2"):
        shared[nm] = np.asarray(inputs[nm], np.float32)
    in_maps = []
    for b in range(x.shape[0]):
        m = dict(shared)
        xt = x[b].T.astype(bf)  # [E, S]
        m["xtb"] = np.ascontiguousarray(
            xt.reshape(EC, P, S // SBA, SBA).transpose(2, 1, 0, 3).reshape(S // SBA, P, EC * SBA)
        )
        in_maps.append(m)
    return in_maps


def kernel(**inputs):
    trivial = all(
        not np.any(np.asarray(inputs[k])) for k in ("bv", "beta1", "beta2", "b1", "b2")
    ) and all(np.all(np.asarray(inputs[k]) == 1.0) for k in ("gamma1", "gamma2"))
    nc, _ = _build(S, trivial)
    in_maps = _prep_in_maps(inputs)
    res = bass_utils.run_bass_kernel_spmd(nc, in_maps, core_ids=list(range(B)))
    out = np.empty((B, S, E), np.float32)
    q = np.empty((B, S, E), np.float32)
    k = np.empty((B, S, E), np.float32)
    v = np.empty((B, S, E), np.float32)
    for b in range(B):
        r = res.results[b]
        out[b] = r["outt_o"].T
        q[b] = r["qt_o"].T
        k[b] = r["kt_o"].T
        v[b] = r["v_o"]
    return (out, k, q, v)
